# revision 20
# baseline (speedup 1.0000x reference)
"""Trainium2 Bass kernel for an 8-batch AttentionBlock (GroupNorm + single-head
self-attention over 64x64 spatial + residual), data-parallel over batch on 8
NeuronCores (one batch element per core).

Per-core math (x: [512, 4096]):
  h   = groupnorm(x) * gamma + beta       (32 groups of 16 ch; h stored fp8)
  u   = (Wk^T Wq) h                       (folded q/k: one fp8 DoubleRow proj)
  s_j = (Wk^T bq).h_j / sqrt(C)           (per-j softmax bias; bk etc. cancel)
  vT  = (Wv h)^T                          (fp8, layout [j, c], direct - no
                                           transposes anywhere in the kernel)
  St  = h^T u          [j, i] blocks, fp8 DoubleRow matmuls into PSUM f32
  Pt  = exp(St*scale + s_j)  fp8 SBUF (ScalarE runs Exp only - no LUT swaps)
  dacc= sum_jb Pt      bf16 SBUF [128, i]  (VectorE + GpSimd split accumulation)
  denb= ones128^T dacc PSUM (all-ones matmul = column-sum broadcast)
  attn= O * recip(denb)  fp8 at 16x true scale (O = vT^T Pt in PSUM; vT
                         carries a 16x host prescale that is NOT divided out)
  y   = (16 Wp) attn  fp8 DoubleRow; out = y/256 + xb fused on VectorE, where
        xb = x + (Wp bv + bp) is host-folded f32, so the residual is exact.

Schedule (the attention St/O fp8-DoubleRow matmuls run at the PE roofline
~216ns per [128,512]xK512 block, so everything else hides behind them):
  phase A: x lands on 2 DMA queues; groupnorm stats split DVE (bn_stats,
           ct0/2) + ScalarE (activation accum, ct1) + GpSimd (tensor_scalar
           accum, ct3); h writes split DVE/ScalarE/GpSimd. h done ~19us.
  phase B: s-bias matmuls batched into one PSUM bank (columns=jb, one copy),
           u[ib0], then the flash loop. ib0 also computes vT (produce-all/
           consume-all, PIPE=16); later ibs pipeline with PIPE=5, computing
           u[ib+1] just-in-time and the previous block's output projection.
"""

import os
import sys

if "/opt/trn_rl_repo" not in sys.path:
    sys.path.insert(0, "/opt/trn_rl_repo")

# recover automatically if a previous run left the NeuronCores wedged
os.environ.setdefault("NEURON_RT_RESET_CORES", "1")

import math

import ml_dtypes
import numpy as np

C = 512
N = 4096
P = 128
CT = C // P      # 4 channel tiles
FB = 512         # free-dim block (i)
NB = N // FB     # 8 i-blocks
JB = N // P      # 32 j-blocks
JP = JB // 2     # 16 j-block pairs (DoubleRow packs 2 k-subtiles)
GS = 16          # channels per group
EPS = 1e-5
PIPE = 5         # jb-pair delay between St/exp emission and den/O consumption

_CACHE = {}


def _build():
    import concourse.tile as tile
    from concourse import bacc, mybir

    f32 = mybir.dt.float32
    bf16 = mybir.dt.bfloat16
    f8 = mybir.dt.float8e4
    AF = mybir.ActivationFunctionType
    ALU = mybir.AluOpType
    DR = mybir.MatmulPerfMode.DoubleRow

    nc = bacc.Bacc("TRN2", target_bir_lowering=False, debug=False, num_devices=8)

    # bf16 copy of x for the groupnorm/stats path (h is fp8 downstream, so
    # bf16 stats are plenty); the exact f32 x only enters via xb (residual).
    xh_d = nc.dram_tensor("xh", [C, N], bf16, kind="ExternalInput").ap()
    # weights arrive pre-tiled as [P, CT, C] so the load is one contiguous DMA.
    # wuT is the folded score matrix (Wk^T Wq, scaled): softmax(q.k) ==
    # softmax(h.(M h) + s[j]) where s[j] = (Wk^T bq).h_j -- bk and the
    # i-only bias terms cancel inside the softmax.
    wu_d = nc.dram_tensor("wuT", [P, CT, C], f8, kind="ExternalInput").ap()
    wv_d = nc.dram_tensor("wvT", [P, CT, C], f8, kind="ExternalInput").ap()
    wp_d = nc.dram_tensor("wpT", [P, CT, C], f8, kind="ExternalInput").ap()
    ws_d = nc.dram_tensor("ws", [P, CT], f8, kind="ExternalInput").ap()
    # xb = x + (Wp bv + bp) per channel, pre-added on host: the entire
    # residual-plus-output-bias term, so the epilogue is one fused op.
    xb_d = nc.dram_tensor("xb", [C, N], f32, kind="ExternalInput").ap()
    gamma_d = nc.dram_tensor("gamma", [P, CT], f32, kind="ExternalInput").ap()
    beta_d = nc.dram_tensor("beta", [P, CT], f32, kind="ExternalInput").ap()
    g16_d = nc.dram_tensor("g16", [P, P // GS], f32, kind="ExternalInput").ap()
    gt_d = nc.dram_tensor("gt", [P // GS, P], f32, kind="ExternalInput").ap()
    out_d = nc.dram_tensor("out", [C, N], f32, kind="ExternalOutput").ap()

    with tile.TileContext(nc) as tc:
        from contextlib import ExitStack

        with ExitStack() as ctx:
            consts = ctx.enter_context(tc.tile_pool(name="consts", bufs=1))
            big = ctx.enter_context(tc.tile_pool(name="big", bufs=1))
            xpool = ctx.enter_context(tc.tile_pool(name="p1", bufs=CT))

            # x feeds the groupnorm critical path - issue its DMAs before
            # anything else lands on the queues (descriptor issue is serial,
            # ~0.6us each). ct0/2 on sync, ct1/3 on scalar: each stats engine
            # gets its tile as early as possible.
            x_tiles = [None] * CT
            for ct in range(CT):
                x_tiles[ct] = xpool.tile([P, N], bf16, name="xt")
            # sync ring: ct0 then ct2 (the DVE bn_stats order); scalar ring:
            # ct1 then ct3's SECOND half first (ScalarE's accum passes need
            # it; DVE picks up ct3's first half last, after ct0/ct2)
            H2 = N // 2
            chunks = {nc.sync: [(0, 0), (0, 1), (2, 0), (2, 1)],
                      nc.scalar: [(1, 0), (1, 1), (3, 1), (3, 0)]}
            for eng, lst in chunks.items():
                for ct, hh in lst:
                    sl = slice(hh * H2, (hh + 1) * H2)
                    eng.dma_start(x_tiles[ct][:, sl],
                                  xh_d[ct * P:(ct + 1) * P, sl])

            def load_w(dram, nm, dt):  # noqa: E306
                t = consts.tile([P, CT, C], dt, name=nm)
                nc.sync.dma_start(t[:], dram)
                return t

            wu_sb = load_w(wu_d, "wu_sb", f8)
            wv_sb = load_w(wv_d, "wv_sb", f8)
            wp_sb = load_w(wp_d, "wp_sb", f8)

            def load_small(dram, shape, nm, dt=f32):
                # gpsimd queue: don't let these tiny loads (needed early by
                # the groupnorm chain) queue behind the big weight DMAs
                t = consts.tile(shape, dt, name=nm)
                nc.gpsimd.dma_start(t[:], dram)
                return t

            ws_sb = load_small(ws_d, [P, CT], "ws_sb", f8)
            gamma_sb = load_small(gamma_d, [P, CT], "gamma_sb")
            beta_sb = load_small(beta_d, [P, CT], "beta_sb")
            g16_sb = load_small(g16_d, [P, P // GS], "g16_sb")
            gt_sb = load_small(gt_d, [P // GS, P], "gt_sb")

            # all-ones weight for the denominator column-sum matmul. vT keeps
            # its 16x fp8 prescale un-divided (attn is stored at 16x true
            # scale, in fp8's sweet spot); wp carries another 16x, and the
            # epilogue divides the combined 256x back out.
            ones128 = consts.tile([P, P], bf16, name="ones128")
            nc.vector.memset(ones128[:], 1.0)
            eps_sb = consts.tile([P // GS, 1], f32, name="eps_sb")
            nc.vector.memset(eps_sb[:], EPS)

            u_sb = big.tile([P, CT, N], f8, name="u")
            vt_sb = big.tile([P, JB, C], f8, name="vt")
            h_sb = big.tile([P, CT, N], f8, name="h")
            # per-j additive softmax bias s[j] (see wuT comment), f32
            st_bias = big.tile([P, JB], f32, name="st_bias")
            attn_sb = big.tile([P, CT, N], f8, name="attn")
            # full-size dummy outs for the accum-based stats paths (one per
            # engine so the passes don't serialize on a shared scratch)
            scr_a = big.tile([P, N], bf16, name="scr_a")
            scr_b = big.tile([P, N], bf16, name="scr_b")

            # shared matmul psum pool (u blocks + St blocks + v blocks)
            sps = ctx.enter_context(tc.tile_pool(name="sps", bufs=2, space="PSUM"))

            # ---------------- phase A: groupnorm -> h ----------------------
            with tc.tile_pool(name="p1s", bufs=2) as p1s, \
                 tc.tile_pool(name="gnps", bufs=1, space="PSUM") as gnps:
                # dummy matmuls warm the PE HAM clock-gate (~3.4us of
                # activity -> 2.4GHz) while the stats chains run; PE would
                # otherwise start the attention matmuls cold
                warm = gnps.tile([P, P], f32, name="warm")

                def warmup(k):
                    for _ in range(k):
                        nc.tensor.matmul(warm[:], lhsT=ones128[:],
                                         rhs=ones128[:], start=True, stop=True)

                warmup(16)
                # stats split: DVE bn_stats for ct0, ct2, and the first half
                # of ct3; ScalarE Identity/Square accum passes for ct1 and the
                # second half of ct3 (I,I,S,S order: one act-table swap).
                # ms_all cols [2ct, 2ct+1] = [mean, E[x^2]] per channel, f32.
                ms_all = p1s.tile([P, 2 * CT], f32, name="ms_all")
                mv_all = p1s.tile([P, 4], f32, name="mv_all")
                mv3 = p1s.tile([P, 2], f32, name="mv3")
                acc3 = p1s.tile([P, 2], f32, name="acc3")
                s2t = p1s.tile([P, 2], f32, name="s2t")
                Nh = N // 2
                for ct in (0, 2):
                    x_t = x_tiles[ct]
                    stats = p1s.tile([P, 8, 6], f32, name="stats")
                    for sg in range(8):
                        nc.vector.bn_stats(
                            stats[:, sg, :], x_t[:, sg * 512:(sg + 1) * 512])
                    nc.vector.bn_aggr(mv_all[:, ct:ct + 2], stats[:])
                    warmup(6)
                stats3 = p1s.tile([P, 4, 6], f32, name="stats3")
                for sg in range(4):
                    nc.vector.bn_stats(
                        stats3[:, sg, :], x_tiles[3][:, sg * 512:(sg + 1) * 512])
                nc.vector.bn_aggr(mv3[:], stats3[:])
                # ScalarE: means first, then squares (batching the act funcs)
                nc.scalar.activation(scr_a[:], x_tiles[1][:], AF.Identity,
                                     bias=0.0, scale=1.0 / N,
                                     accum_out=ms_all[:, 2:3])
                nc.scalar.activation(scr_b[:, :Nh], x_tiles[3][:, Nh:],
                                     AF.Identity, bias=0.0, scale=1.0 / Nh,
                                     accum_out=acc3[:, 0:1])
                nc.scalar.activation(scr_a[:], x_tiles[1][:], AF.Square,
                                     bias=0.0, scale=1.0,
                                     accum_out=s2t[:, 0:1])
                nc.scalar.activation(scr_b[:, :Nh], x_tiles[3][:, Nh:],
                                     AF.Square, bias=0.0, scale=1.0,
                                     accum_out=s2t[:, 1:2])
                warmup(8)
                # assemble ms_all: bn cts (strided), ct1 E[x^2], ct3 combine
                gmt = p1s.tile([P, 2], f32, name="gmt")
                nc.gpsimd.tensor_copy(ms_all[:, 0::4], mv_all[:, 0::2])
                nc.gpsimd.tensor_mul(gmt[:], mv_all[:, 0::2], mv_all[:, 0::2])
                nc.gpsimd.tensor_add(ms_all[:, 1::4], mv_all[:, 1::2], gmt[:])
                nc.gpsimd.tensor_scalar(out=ms_all[:, 3:4], in0=s2t[:, 0:1],
                                        scalar1=1.0 / N, scalar2=None,
                                        op0=ALU.mult)
                # ct3: half a from bn (mean_a, var_a), half b from accums
                e2a = p1s.tile([P, 1], f32, name="e2a")
                nc.vector.tensor_mul(e2a[:], mv3[:, 0:1], mv3[:, 0:1])
                nc.vector.tensor_add(e2a[:], e2a[:], mv3[:, 1:2])
                nc.vector.tensor_add(ms_all[:, 6:7], mv3[:, 0:1], acc3[:, 0:1])
                nc.vector.tensor_scalar(out=ms_all[:, 6:7], in0=ms_all[:, 6:7],
                                        scalar1=0.5, scalar2=None, op0=ALU.mult)
                nc.vector.tensor_scalar(out=e2a[:], in0=e2a[:], scalar1=0.5,
                                        scalar2=None, op0=ALU.mult)
                nc.vector.tensor_scalar(out=ms_all[:, 7:8], in0=s2t[:, 1:2],
                                        scalar1=0.5 / Nh, scalar2=None,
                                        op0=ALU.mult)
                nc.vector.tensor_add(ms_all[:, 7:8], ms_all[:, 7:8], e2a[:])

                # one batched group-norm chain for all 4 cts ([8, 2] per ct)
                gps = gnps.tile([P // GS, 2 * CT], f32, name="gps")
                nc.tensor.matmul(gps[:], lhsT=g16_sb[:], rhs=ms_all[:],
                                 start=True, stop=True)
                gsb = p1s.tile([P // GS, 2 * CT], f32, name="gsb")
                nc.vector.tensor_copy(gsb[:], gps[:])
                gm2 = p1s.tile([P // GS, CT], f32, name="gm2")
                nc.gpsimd.tensor_mul(gm2[:], gsb[:, 0::2], gsb[:, 0::2])
                nc.gpsimd.tensor_sub(gsb[:, 1::2], gsb[:, 1::2], gm2[:])
                # var cols -> 1/sqrt(var + eps)
                nc.scalar.activation(gsb[:, 1::2], gsb[:, 1::2], AF.Sqrt,
                                     bias=eps_sb[:], scale=1.0)
                # dummy exp: swap the act table to the exp set now, while the
                # h writes / u copies run, so the first real exp doesn't pay
                # the ~1.3us LUT load (identity lives in every set)
                nc.scalar.activation(s2t[:, 0:1], s2t[:, 0:1], AF.Exp,
                                     bias=0.0, scale=0.0)
                nc.vector.reciprocal_approx_fast(gsb[:, 1::2], gsb[:, 1::2])
                # broadcast group (mean, rstd) back to 128 channels
                cps = gnps.tile([P, 2 * CT], f32, name="cps")
                nc.tensor.matmul(cps[:], lhsT=gt_sb[:], rhs=gsb[:],
                                 start=True, stop=True)
                scale_all = p1s.tile([P, CT], f32, name="scale_all")
                nc.vector.tensor_mul(scale_all[:], cps[:, 1::2], gamma_sb[:])
                nb1 = p1s.tile([P, CT], f32, name="nb1")
                nc.vector.tensor_mul(nb1[:], cps[:, 0::2], scale_all[:])
                nbias_all = p1s.tile([P, CT], f32, name="nbias_all")
                nc.vector.tensor_sub(nbias_all[:], beta_sb[:], nb1[:])
                warmup(6)
                # h writes, spread across DVE / GpSimd / ScalarE
                h_engs = (nc.vector, nc.gpsimd, nc.scalar, nc.vector,
                          nc.gpsimd, nc.scalar, nc.vector, nc.gpsimd)
                for ct in range(CT):
                    x_t = x_tiles[ct]
                    for hh in range(2):
                        sl = slice(hh * (N // 2), (hh + 1) * (N // 2))
                        eng = h_engs[2 * ct + hh]
                        if eng is nc.scalar:
                            nc.scalar.activation(
                                h_sb[:, ct, sl], x_t[:, sl], AF.Identity,
                                bias=nbias_all[:, ct:ct + 1],
                                scale=scale_all[:, ct:ct + 1])
                        else:
                            eng.tensor_scalar(
                                out=h_sb[:, ct, sl], in0=x_t[:, sl],
                                scalar1=scale_all[:, ct:ct + 1],
                                scalar2=nbias_all[:, ct:ct + 1],
                                op0=ALU.mult, op1=ALU.add)
                    warmup(4)

            # ---------------- phase B: attention -----------------------------
            with tc.tile_pool(name="ptpool", bufs=JP + 2) as ptp, \
                 tc.tile_pool(name="ops", bufs=1, space="PSUM") as ops, \
                 tc.tile_pool(name="dps", bufs=2, space="PSUM") as dps, \
                 tc.tile_pool(name="dpool", bufs=2) as dpool, \
                 tc.tile_pool(name="mpool", bufs=2) as mp, \
                 tc.tile_pool(name="xrpool", bufs=5) as xrp, \
                 tc.tile_pool(name="outpool", bufs=3) as outp:

                # s-bias scratch: one PSUM bank (shares the o3 bank; ib0's
                # o_tiles are allocated only at first consume, after all
                # s matmuls have been copied out), one column per j-block.
                # Full-bank shape so every o3 allocation is the same size.
                sp = ops.tile([P, FB], f32, name="o3")

                def u_ct(ib, ct):
                    # one [128, FB] chunk of u[:, :, ib-block]: 2 DoubleRow
                    # matmuls + a PSUM copy. The copy goes on VectorE: on
                    # ScalarE it would delay exps, which gate the St psum
                    # rotation (sps bufs=2)
                    qp = sps.tile([P, FB], f32, name="st")
                    for kt in range(0, CT, 2):
                        nc.tensor.matmul(
                            qp[:],
                            lhsT=wu_sb[:, kt:kt + 2, ct * P:(ct + 1) * P],
                            rhs=h_sb[:, kt:kt + 2, ib * FB:(ib + 1) * FB],
                            start=(kt == 0), stop=(kt == CT - 2),
                            perf_mode=DR)
                    nc.vector.tensor_copy(
                        u_sb[:, ct, ib * FB:(ib + 1) * FB], qp[:])

                for ct in range(CT):
                    u_ct(0, ct)

                def final_proj(ib):
                    xrs = []
                    for ct in range(CT):
                        xr = xrp.tile([P, FB], f32, name="xr")
                        nc.sync.dma_start(
                            xr[:], xb_d[ct * P:(ct + 1) * P, ib * FB:(ib + 1) * FB])
                        xrs.append(xr)
                    for ct in range(CT):
                        yp = dps.tile([P, FB], f32, name="scr")
                        for kt in range(0, CT, 2):
                            nc.tensor.matmul(
                                yp[:],
                                lhsT=wp_sb[:, kt:kt + 2, ct * P:(ct + 1) * P],
                                rhs=attn_sb[:, kt:kt + 2, ib * FB:(ib + 1) * FB],
                                start=(kt == 0), stop=(kt == CT - 2),
                                perf_mode=DR)
                        ot = outp.tile([P, FB], f32, name="ot")
                        # y/256 + xb fused: undoes the 16x on vT and 16x on wp
                        nc.vector.scalar_tensor_tensor(
                            out=ot[:], in0=yp[:], scalar=1.0 / 256.0,
                            in1=xrs[ct][:], op0=ALU.mult, op1=ALU.add)
                        nc.sync.dma_start(
                            out_d[ct * P:(ct + 1) * P, ib * FB:(ib + 1) * FB],
                            ot[:])

                for ib in range(NB):
                    o_tiles = []
                    # two independent denominator accumulators halve the
                    # serial DVE chain; bf16 is plenty (errors average out
                    # 1/sqrt(128) in the column-sum matmul) and keeps the den
                    # matmul single-pass
                    dacc = [dpool.tile([P, FB], bf16, name=f"dacc{h}")
                            for h in range(2)]
                    pt_q = []

                    def consume(jp, pt):
                        if jp == 0:
                            # lazy: ib0's vT/s matmuls borrow these banks
                            # during its produce phase
                            o_tiles.extend(ops.tile([P, FB], f32, name=f"o{cs}")
                                           for cs in range(CT))
                        # accumulator 0 on VectorE, accumulator 1 on the
                        # otherwise-idle GpSimd so neither serial chain gates
                        # pt-tile reuse
                        for h, eng in ((0, nc.vector), (1, nc.gpsimd)):
                            if jp == 0:
                                eng.tensor_copy(dacc[h][:], pt[:, h, :])
                            else:
                                eng.tensor_add(dacc[h][:], dacc[h][:],
                                               pt[:, h, :])
                        for cs in range(CT):
                            nc.tensor.matmul(
                                o_tiles[cs][:],
                                lhsT=vt_sb[:, 2 * jp:2 * jp + 2,
                                           cs * P:(cs + 1) * P],
                                rhs=pt[:],
                                start=(jp == 0), stop=(jp == JP - 1),
                                perf_mode=DR)

                    # ib0 also computes vT and the s bias (both share h_j
                    # stationary tiles with St and must lead the O
                    # consumption), so it produces all 16 pairs first and
                    # consumes after; later ibs pipeline with PIPE=5.
                    pipe = JP if ib == 0 else PIPE
                    for jp in range(JP):
                        pt = ptp.tile([P, 2, FB], f8, name="pt")
                        for h in range(2):
                            jb = 2 * jp + h
                            if ib == 0:
                                # v_j = Wv h_j, transposed layout [j, c];
                                # psum rotates over the idle o0/o1/o2 banks
                                vp = ops.tile([P, C], f32, name=f"o{jb % 3}")
                                for kt in range(0, CT, 2):
                                    nc.tensor.matmul(
                                        vp[:],
                                        lhsT=h_sb[:, kt:kt + 2,
                                                  jb * P:(jb + 1) * P],
                                        rhs=wv_sb[:, kt:kt + 2, :],
                                        start=(kt == 0), stop=(kt == CT - 2),
                                        perf_mode=DR)
                                nc.vector.tensor_copy(vt_sb[:, jb, :], vp[:])
                                # s[jb] into column jb of the shared sp bank;
                                # ws carries a 128x host scale (fp8 guard)
                                for kt in range(0, CT, 2):
                                    nc.tensor.matmul(
                                        sp[:, jb:jb + 1],
                                        lhsT=h_sb[:, kt:kt + 2,
                                                  jb * P:(jb + 1) * P],
                                        rhs=ws_sb[:, kt:kt + 2, None],
                                        start=(kt == 0), stop=(kt == CT - 2),
                                        perf_mode=DR, skip_group_check=True)
                                nc.vector.tensor_scalar(
                                    out=st_bias[:, jb:jb + 1],
                                    in0=sp[:, jb:jb + 1],
                                    scalar1=1.0 / 128.0, scalar2=None,
                                    op0=ALU.mult)
                            st = sps.tile([P, FB], f32, name="st")
                            for kt in range(0, CT, 2):
                                nc.tensor.matmul(
                                    st[:],
                                    lhsT=h_sb[:, kt:kt + 2,
                                              jb * P:(jb + 1) * P],
                                    rhs=u_sb[:, kt:kt + 2,
                                             ib * FB:(ib + 1) * FB],
                                    start=(kt == 0), stop=(kt == CT - 2),
                                    perf_mode=DR)
                            # wuT carries a 32x host scale; undo it plus the
                            # 1/sqrt(C) attention scale inside the exp, and add
                            # the per-j softmax bias s[j]
                            nc.scalar.activation(pt[:, h, :], st[:], AF.Exp,
                                                 bias=st_bias[:, jb:jb + 1],
                                                 scale=1.0 / (32.0 * math.sqrt(C)))
                        pt_q.append((jp, pt))
                        if ib > 0:
                            if jp == PIPE:
                                # overlap previous block's output projection
                                # with this block's score matmuls
                                final_proj(ib - 1)
                            if ib < NB - 1 and jp in (8, 10, 12, 14):
                                # next block's u, just-in-time, spread out
                                u_ct(ib + 1, (jp - 8) // 2)
                        if jp >= pipe:
                            consume(*pt_q.pop(0))
                    while pt_q:
                        jp_, pt_ = pt_q.pop(0)
                        consume(jp_, pt_)
                        if ib == 0 and NB > 1 and jp_ in (3, 6, 9, 12):
                            u_ct(1, (jp_ // 3) - 1)

                    # all-ones matmul: every psum partition gets sum_j dacc[j,:]
                    denb = dps.tile([P, FB], f32, name="scr")
                    nc.tensor.matmul(denb[:], lhsT=ones128[:], rhs=dacc[0][:],
                                     start=True, stop=False)
                    nc.tensor.matmul(denb[:], lhsT=ones128[:], rhs=dacc[1][:],
                                     start=False, stop=True)
                    rdb = mp.tile([P, FB], f32, name="rdb")
                    nc.vector.reciprocal_approx_fast(rdb[:], denb[:])
                    for cs in range(CT):
                        nc.vector.tensor_mul(
                            attn_sb[:, cs, ib * FB:(ib + 1) * FB],
                            o_tiles[cs][:], rdb[:])
                final_proj(NB - 1)

    nc.compile()
    return nc


def _host_inputs(x, gamma, beta, Wq, bq, Wk, bk, Wv, bv, Wp, bp):
    bf16 = ml_dtypes.bfloat16
    f32 = np.float32
    B = x.shape[0]
    xs = np.asarray(x, f32).reshape(B, C, N)

    def fold(v):
        return np.asarray(v, f32).reshape(CT, P).T.copy()

    f8 = ml_dtypes.float8_e4m3fn

    def wtile(w, scale, dt):
        # [Cout, Cin] -> transposed [Cin, Cout] -> tiled [P, CT, Cout]
        wT = np.asarray(w, f32).T * scale
        return np.ascontiguousarray(
            wT.reshape(CT, P, C).transpose(1, 0, 2)).astype(dt)

    # folded score matrix: softmax_j(q_i.k_j/sqrt(C)) with q=Wq h+bq,
    # k=Wk h+bk equals softmax_j(h_j.(M h_i)/sqrt(C) + s_j) with
    # M = Wk^T Wq and s = (Wk^T bq).h_j/sqrt(C); bk and i-only terms cancel.
    M = np.asarray(Wk, f32).T @ np.asarray(Wq, f32)
    wsv = (np.asarray(Wk, f32).T @ np.asarray(bq, f32)) / math.sqrt(C)
    common = {
        "wuT": wtile(M, 32.0, f8),
        "wvT": wtile(Wv, 16.0, f8),
        "wpT": wtile(Wp, 16.0, f8),
        "ws": (wsv * 128.0).reshape(CT, P).T.copy().astype(f8),
        "gamma": fold(gamma),
        "beta": fold(beta),
    }
    bias_out = (np.asarray(Wp, f32) @ np.asarray(bv, f32)
                + np.asarray(bp, f32)).astype(f32)
    xbs = xs + bias_out[None, :, None]
    g16 = np.zeros((P, P // GS), f32)
    g16[np.arange(P), np.arange(P) // GS] = 1.0 / GS
    gt = np.zeros((P // GS, P), f32)
    gt[np.arange(P) // GS, np.arange(P)] = 1.0
    common["g16"] = g16
    common["gt"] = gt
    return [dict(common, xh=np.ascontiguousarray(xs[b]).astype(bf16),
                 xb=np.ascontiguousarray(xbs[b])) for b in range(B)]


def kernel(x, gamma, beta, Wq, bq, Wk, bk, Wv, bv, Wp, bp, _trace=False):
    from concourse.bass_utils import run_bass_kernel_spmd

    if "nc" not in _CACHE:
        _CACHE["nc"] = _build()
    nc = _CACHE["nc"]
    in_maps = _host_inputs(x, gamma, beta, Wq, bq, Wk, bk, Wv, bv, Wp, bp)
    B = len(in_maps)
    res = run_bass_kernel_spmd(nc, in_maps, core_ids=list(range(B)),
                               trace=_trace)
    out = np.stack([res.results[b]["out"] for b in range(B)])
    out = out.reshape(x.shape).astype(np.float32)
    if _trace:
        _CACHE["last_results"] = res
    return out


# revision 26
# speedup vs baseline: 1.0492x; 1.0492x over previous
"""Trainium2 Bass kernel for an 8-batch AttentionBlock (GroupNorm + single-head
self-attention over 64x64 spatial + residual), data-parallel over batch on 8
NeuronCores (one batch element per core).

Per-core math (x: [512, 4096]):
  h   = groupnorm(x) * gamma + beta       (32 groups of 16 ch; h stored fp8)
  u   = (Wk^T Wq) h                       (folded q/k: one fp8 DoubleRow proj)
  s_j = (Wk^T bq).h_j / sqrt(C)           (per-j softmax bias; bk etc. cancel)
  vT  = (Wv h)^T                          (fp8, layout [j, c], direct - no
                                           transposes anywhere in the kernel)
  St  = h^T u          [j, i] blocks, fp8 DoubleRow matmuls into PSUM f32
  Pt  = exp(St*scale + s_j)  fp8 SBUF (ScalarE runs Exp only - no LUT swaps)
  dacc= sum_jb Pt      bf16 SBUF [128, i]  (VectorE + GpSimd split accumulation)
  denb= ones128^T dacc PSUM (all-ones matmul = column-sum broadcast)
  attn= O * recip(denb)  fp8 at 16x true scale (O = vT^T Pt in PSUM; vT
                         carries a 16x host prescale that is NOT divided out)
  y   = (16 Wp) attn  fp8 DoubleRow; out = y/256 + xb fused on VectorE, where
        xb = x + (Wp bv + bp) is host-folded f32, so the residual is exact.

Schedule (the attention St/O fp8-DoubleRow matmuls run at the PE roofline
~216ns per [128,512]xK512 block, so everything else hides behind them):
  phase A: x lands on 2 DMA queues; groupnorm stats split DVE (bn_stats,
           ct0/2) + ScalarE (activation accum, ct1) + GpSimd (tensor_scalar
           accum, ct3); h writes split DVE/ScalarE/GpSimd. h done ~19us.
  phase B: s-bias matmuls batched into one PSUM bank (columns=jb, one copy),
           u[ib0], then the flash loop. ib0 also computes vT (produce-all/
           consume-all, PIPE=16); later ibs pipeline with PIPE=5, computing
           u[ib+1] just-in-time and the previous block's output projection.
"""

import os
import sys

if "/opt/trn_rl_repo" not in sys.path:
    sys.path.insert(0, "/opt/trn_rl_repo")

# recover automatically if a previous run left the NeuronCores wedged
os.environ.setdefault("NEURON_RT_RESET_CORES", "1")

import math

import ml_dtypes
import numpy as np

C = 512
N = 4096
P = 128
CT = C // P      # 4 channel tiles
FB = 512         # free-dim block (i)
NB = N // FB     # 8 i-blocks
JB = N // P      # 32 j-blocks
JP = JB // 2     # 16 j-block pairs (DoubleRow packs 2 k-subtiles)
GS = 16          # channels per group
EPS = 1e-5
PIPE = 5         # jb-pair delay between St/exp emission and den/O consumption

_CACHE = {}


def _build():
    import concourse.tile as tile
    from concourse import bacc, mybir

    f32 = mybir.dt.float32
    bf16 = mybir.dt.bfloat16
    f8 = mybir.dt.float8e4
    AF = mybir.ActivationFunctionType
    ALU = mybir.AluOpType
    DR = mybir.MatmulPerfMode.DoubleRow

    nc = bacc.Bacc("TRN2", target_bir_lowering=False, debug=False, num_devices=8)

    # bf16 copy of x for the groupnorm/stats path (h is fp8 downstream, so
    # bf16 stats are plenty); the exact f32 x only enters via xb (residual).
    xh_d = nc.dram_tensor("xh", [C, N], bf16, kind="ExternalInput").ap()
    # weights arrive pre-tiled as [P, CT, C] so the load is one contiguous DMA.
    # wuT is the folded score matrix (Wk^T Wq, scaled): softmax(q.k) ==
    # softmax(h.(M h) + s[j]) where s[j] = (Wk^T bq).h_j -- bk and the
    # i-only bias terms cancel inside the softmax.
    wu_d = nc.dram_tensor("wuT", [P, CT, C], f8, kind="ExternalInput").ap()
    wv_d = nc.dram_tensor("wvT", [P, CT, C], f8, kind="ExternalInput").ap()
    wp_d = nc.dram_tensor("wpT", [P, CT, C], f8, kind="ExternalInput").ap()
    ws_d = nc.dram_tensor("ws", [P, CT], f8, kind="ExternalInput").ap()
    # xb = x + (Wp bv + bp) per channel, pre-added on host: the entire
    # residual-plus-output-bias term, so the epilogue is one fused op.
    xb_d = nc.dram_tensor("xb", [C, N], f32, kind="ExternalInput").ap()
    gamma_d = nc.dram_tensor("gamma", [P, CT], f32, kind="ExternalInput").ap()
    beta_d = nc.dram_tensor("beta", [P, CT], f32, kind="ExternalInput").ap()
    g16_d = nc.dram_tensor("g16", [P, P // GS], f32, kind="ExternalInput").ap()
    gt_d = nc.dram_tensor("gt", [P // GS, P], f32, kind="ExternalInput").ap()
    out_d = nc.dram_tensor("out", [C, N], f32, kind="ExternalOutput").ap()

    with tile.TileContext(nc) as tc:
        from contextlib import ExitStack

        with ExitStack() as ctx:
            consts = ctx.enter_context(tc.tile_pool(name="consts", bufs=1))
            big = ctx.enter_context(tc.tile_pool(name="big", bufs=1))
            xpool = ctx.enter_context(tc.tile_pool(name="p1", bufs=CT))

            # x feeds the groupnorm critical path - issue its DMAs before
            # anything else lands on the queues (descriptor issue is serial,
            # ~0.6us each). ct0/2 on sync, ct1/3 on scalar: each stats engine
            # gets its tile as early as possible.
            x_tiles = [None] * CT
            for ct in range(CT):
                x_tiles[ct] = xpool.tile([P, N], bf16, name="xt")
            # sync ring: ct0 then ct2 (the DVE bn_stats order); scalar ring:
            # ct1 then ct3's SECOND half first (ScalarE's accum passes need
            # it; DVE picks up ct3's first half last, after ct0/ct2)
            H2 = N // 2
            chunks = {nc.sync: [(0, 0), (0, 1), (2, 0), (2, 1)],
                      nc.scalar: [(1, 0), (1, 1), (3, 1), (3, 0)]}
            for eng, lst in chunks.items():
                for ct, hh in lst:
                    sl = slice(hh * H2, (hh + 1) * H2)
                    eng.dma_start(x_tiles[ct][:, sl],
                                  xh_d[ct * P:(ct + 1) * P, sl])

            def load_w(dram, nm, dt):  # noqa: E306
                t = consts.tile([P, CT, C], dt, name=nm)
                nc.sync.dma_start(t[:], dram)
                return t

            wu_sb = load_w(wu_d, "wu_sb", f8)
            wv_sb = load_w(wv_d, "wv_sb", f8)
            wp_sb = load_w(wp_d, "wp_sb", f8)

            def load_small(dram, shape, nm, dt=f32):
                # gpsimd queue: don't let these tiny loads (needed early by
                # the groupnorm chain) queue behind the big weight DMAs
                t = consts.tile(shape, dt, name=nm)
                nc.gpsimd.dma_start(t[:], dram)
                return t

            ws_sb = load_small(ws_d, [P, CT], "ws_sb", f8)
            gamma_sb = load_small(gamma_d, [P, CT], "gamma_sb")
            beta_sb = load_small(beta_d, [P, CT], "beta_sb")
            g16_sb = load_small(g16_d, [P, P // GS], "g16_sb")
            gt_sb = load_small(gt_d, [P // GS, P], "gt_sb")

            # all-ones weight for the denominator column-sum matmul. vT keeps
            # its 16x fp8 prescale un-divided (attn is stored at 16x true
            # scale, in fp8's sweet spot); wp carries another 16x, and the
            # epilogue divides the combined 256x back out.
            ones128 = consts.tile([P, P], bf16, name="ones128")
            nc.vector.memset(ones128[:], 1.0)
            eps_sb = consts.tile([P // GS, 1], f32, name="eps_sb")
            nc.vector.memset(eps_sb[:], EPS)

            u_sb = big.tile([P, CT, N], f8, name="u")
            vt_sb = big.tile([P, JB, C], f8, name="vt")
            h_sb = big.tile([P, CT, N], f8, name="h")
            # per-j additive softmax bias s[j] (see wuT comment), f32
            st_bias = big.tile([P, JB], f32, name="st_bias")
            attn_sb = big.tile([P, CT, N], f8, name="attn")
            # full-size dummy outs for the accum-based stats paths (one per
            # engine so the passes don't serialize on a shared scratch)
            scr_a = big.tile([P, N], bf16, name="scr_a")
            scr_b = big.tile([P, N], bf16, name="scr_b")

            # shared matmul psum pool (u blocks + St blocks + v blocks)
            sps = ctx.enter_context(tc.tile_pool(name="sps", bufs=2, space="PSUM"))

            # ---------------- phase A: groupnorm -> h ----------------------
            with tc.tile_pool(name="p1s", bufs=2) as p1s, \
                 tc.tile_pool(name="gnps", bufs=1, space="PSUM") as gnps:
                # dummy matmuls warm the PE HAM clock-gate (~3.4us of
                # activity -> 2.4GHz) while the stats chains run; PE would
                # otherwise start the attention matmuls cold
                warm = gnps.tile([P, P], f32, name="warm")

                def warmup(k):
                    for _ in range(k):
                        nc.tensor.matmul(warm[:], lhsT=ones128[:],
                                         rhs=ones128[:], start=True, stop=True)

                warmup(16)
                # stats split: DVE bn_stats for ct0, ct2, and the first half
                # of ct3; ScalarE Identity/Square accum passes for ct1 and the
                # second half of ct3 (I,I,S,S order: one act-table swap).
                # ms_all cols [2ct, 2ct+1] = [mean, E[x^2]] per channel, f32.
                ms_all = p1s.tile([P, 2 * CT], f32, name="ms_all")
                mv_all = p1s.tile([P, 4], f32, name="mv_all")
                mv3 = p1s.tile([P, 2], f32, name="mv3")
                acc3 = p1s.tile([P, 2], f32, name="acc3")
                s2t = p1s.tile([P, 2], f32, name="s2t")
                Nh = N // 2
                for ct in (0, 2):
                    x_t = x_tiles[ct]
                    stats = p1s.tile([P, 8, 6], f32, name="stats")
                    for sg in range(8):
                        nc.vector.bn_stats(
                            stats[:, sg, :], x_t[:, sg * 512:(sg + 1) * 512])
                    nc.vector.bn_aggr(mv_all[:, ct:ct + 2], stats[:])
                    warmup(6)
                stats3 = p1s.tile([P, 4, 6], f32, name="stats3")
                for sg in range(4):
                    nc.vector.bn_stats(
                        stats3[:, sg, :], x_tiles[3][:, sg * 512:(sg + 1) * 512])
                nc.vector.bn_aggr(mv3[:], stats3[:])
                # ScalarE: means first, then squares (batching the act funcs)
                nc.scalar.activation(scr_a[:], x_tiles[1][:], AF.Identity,
                                     bias=0.0, scale=1.0 / N,
                                     accum_out=ms_all[:, 2:3])
                nc.scalar.activation(scr_b[:, :Nh], x_tiles[3][:, Nh:],
                                     AF.Identity, bias=0.0, scale=1.0 / Nh,
                                     accum_out=acc3[:, 0:1])
                nc.scalar.activation(scr_a[:], x_tiles[1][:], AF.Square,
                                     bias=0.0, scale=1.0,
                                     accum_out=s2t[:, 0:1])
                nc.scalar.activation(scr_b[:, :Nh], x_tiles[3][:, Nh:],
                                     AF.Square, bias=0.0, scale=1.0,
                                     accum_out=s2t[:, 1:2])
                warmup(8)
                # assemble ms_all: bn cts (strided), ct1 E[x^2], ct3 combine
                gmt = p1s.tile([P, 2], f32, name="gmt")
                nc.gpsimd.tensor_copy(ms_all[:, 0::4], mv_all[:, 0::2])
                nc.gpsimd.tensor_mul(gmt[:], mv_all[:, 0::2], mv_all[:, 0::2])
                nc.gpsimd.tensor_add(ms_all[:, 1::4], mv_all[:, 1::2], gmt[:])
                nc.gpsimd.tensor_scalar(out=ms_all[:, 3:4], in0=s2t[:, 0:1],
                                        scalar1=1.0 / N, scalar2=None,
                                        op0=ALU.mult)
                # ct3: half a from bn (mean_a, var_a), half b from accums
                e2a = p1s.tile([P, 1], f32, name="e2a")
                nc.vector.tensor_mul(e2a[:], mv3[:, 0:1], mv3[:, 0:1])
                nc.vector.tensor_add(e2a[:], e2a[:], mv3[:, 1:2])
                nc.vector.tensor_add(ms_all[:, 6:7], mv3[:, 0:1], acc3[:, 0:1])
                nc.vector.tensor_scalar(out=ms_all[:, 6:7], in0=ms_all[:, 6:7],
                                        scalar1=0.5, scalar2=None, op0=ALU.mult)
                nc.vector.tensor_scalar(out=e2a[:], in0=e2a[:], scalar1=0.5,
                                        scalar2=None, op0=ALU.mult)
                nc.vector.tensor_scalar(out=ms_all[:, 7:8], in0=s2t[:, 1:2],
                                        scalar1=0.5 / Nh, scalar2=None,
                                        op0=ALU.mult)
                nc.vector.tensor_add(ms_all[:, 7:8], ms_all[:, 7:8], e2a[:])

                # one batched group-norm chain for all 4 cts ([8, 2] per ct)
                gps = gnps.tile([P // GS, 2 * CT], f32, name="gps")
                nc.tensor.matmul(gps[:], lhsT=g16_sb[:], rhs=ms_all[:],
                                 start=True, stop=True)
                gsb = p1s.tile([P // GS, 2 * CT], f32, name="gsb")
                nc.vector.tensor_copy(gsb[:], gps[:])
                gm2 = p1s.tile([P // GS, CT], f32, name="gm2")
                nc.gpsimd.tensor_mul(gm2[:], gsb[:, 0::2], gsb[:, 0::2])
                nc.gpsimd.tensor_sub(gsb[:, 1::2], gsb[:, 1::2], gm2[:])
                # var cols -> 1/sqrt(var + eps)
                nc.scalar.activation(gsb[:, 1::2], gsb[:, 1::2], AF.Sqrt,
                                     bias=eps_sb[:], scale=1.0)
                # dummy exp: swap the act table to the exp set now, while the
                # h writes / u copies run, so the first real exp doesn't pay
                # the ~1.3us LUT load (identity lives in every set)
                nc.scalar.activation(s2t[:, 0:1], s2t[:, 0:1], AF.Exp,
                                     bias=0.0, scale=0.0)
                nc.vector.reciprocal_approx_fast(gsb[:, 1::2], gsb[:, 1::2])
                # broadcast group (mean, rstd) back to 128 channels
                cps = gnps.tile([P, 2 * CT], f32, name="cps")
                nc.tensor.matmul(cps[:], lhsT=gt_sb[:], rhs=gsb[:],
                                 start=True, stop=True)
                scale_all = p1s.tile([P, CT], f32, name="scale_all")
                nc.vector.tensor_mul(scale_all[:], cps[:, 1::2], gamma_sb[:])
                nb1 = p1s.tile([P, CT], f32, name="nb1")
                nc.vector.tensor_mul(nb1[:], cps[:, 0::2], scale_all[:])
                nbias_all = p1s.tile([P, CT], f32, name="nbias_all")
                nc.vector.tensor_sub(nbias_all[:], beta_sb[:], nb1[:])
                warmup(6)
                # h writes, spread across DVE / GpSimd / ScalarE (DVE is
                # fastest per chunk, so it takes 4 of the 8)
                h_engs = (nc.vector, nc.vector, nc.scalar, nc.gpsimd,
                          nc.vector, nc.scalar, nc.gpsimd, nc.vector)
                for ct in range(CT):
                    x_t = x_tiles[ct]
                    for hh in range(2):
                        sl = slice(hh * (N // 2), (hh + 1) * (N // 2))
                        eng = h_engs[2 * ct + hh]
                        if eng is nc.scalar:
                            nc.scalar.activation(
                                h_sb[:, ct, sl], x_t[:, sl], AF.Identity,
                                bias=nbias_all[:, ct:ct + 1],
                                scale=scale_all[:, ct:ct + 1])
                        else:
                            eng.tensor_scalar(
                                out=h_sb[:, ct, sl], in0=x_t[:, sl],
                                scalar1=scale_all[:, ct:ct + 1],
                                scalar2=nbias_all[:, ct:ct + 1],
                                op0=ALU.mult, op1=ALU.add)
                    warmup(4)

            # ---------------- phase B: attention -----------------------------
            with tc.tile_pool(name="ptpool", bufs=JP + 2) as ptp, \
                 tc.tile_pool(name="ops", bufs=1, space="PSUM") as ops, \
                 tc.tile_pool(name="dps", bufs=2, space="PSUM") as dps, \
                 tc.tile_pool(name="dpool", bufs=2) as dpool, \
                 tc.tile_pool(name="mpool", bufs=2) as mp, \
                 tc.tile_pool(name="xrpool", bufs=5) as xrp, \
                 tc.tile_pool(name="outpool", bufs=3) as outp:

                # s-bias scratch: one PSUM bank (shares the o3 bank; ib0's
                # o_tiles are allocated only at first consume, after all
                # s matmuls have been copied out), one column per j-block.
                # Full-bank shape so every o3 allocation is the same size.
                sp = ops.tile([P, FB], f32, name="o3")

                def u_ct(ib, ct, pool, copy_eng):
                    # one [128, FB] chunk of u[:, :, ib-block]: 2 DoubleRow
                    # matmuls + a PSUM copy. JIT chunks borrow the dps 'scr'
                    # banks (idle between final-proj bursts) so the St psum
                    # rotation in sps never waits on a u copy.
                    qp = pool.tile([P, FB], f32,
                                   name="st" if pool is sps else "scr")
                    for kt in range(0, CT, 2):
                        nc.tensor.matmul(
                            qp[:],
                            lhsT=wu_sb[:, kt:kt + 2, ct * P:(ct + 1) * P],
                            rhs=h_sb[:, kt:kt + 2, ib * FB:(ib + 1) * FB],
                            start=(kt == 0), stop=(kt == CT - 2),
                            perf_mode=DR)
                    if copy_eng is nc.scalar:
                        nc.scalar.activation(
                            u_sb[:, ct, ib * FB:(ib + 1) * FB], qp[:],
                            AF.Identity, bias=0.0, scale=1.0)
                    else:
                        copy_eng.tensor_copy(
                            u_sb[:, ct, ib * FB:(ib + 1) * FB], qp[:])

                for ct in range(CT):
                    u_ct(0, ct, sps, nc.vector)

                def final_proj(ib):
                    xrs = []
                    for ct in range(CT):
                        xr = xrp.tile([P, FB], f32, name="xr")
                        nc.sync.dma_start(
                            xr[:], xb_d[ct * P:(ct + 1) * P, ib * FB:(ib + 1) * FB])
                        xrs.append(xr)
                    for ct in range(CT):
                        yp = dps.tile([P, FB], f32, name="scr")
                        for kt in range(0, CT, 2):
                            nc.tensor.matmul(
                                yp[:],
                                lhsT=wp_sb[:, kt:kt + 2, ct * P:(ct + 1) * P],
                                rhs=attn_sb[:, kt:kt + 2, ib * FB:(ib + 1) * FB],
                                start=(kt == 0), stop=(kt == CT - 2),
                                perf_mode=DR)
                        ot = outp.tile([P, FB], f32, name="ot")
                        # y/256 + xb fused: undoes the 16x on vT and 16x on wp
                        nc.vector.scalar_tensor_tensor(
                            out=ot[:], in0=yp[:], scalar=1.0 / 256.0,
                            in1=xrs[ct][:], op0=ALU.mult, op1=ALU.add)
                        nc.sync.dma_start(
                            out_d[ct * P:(ct + 1) * P, ib * FB:(ib + 1) * FB],
                            ot[:])

                for ib in range(NB):
                    o_tiles = []
                    # two independent denominator accumulators halve the
                    # serial DVE chain; bf16 is plenty (errors average out
                    # 1/sqrt(128) in the column-sum matmul) and keeps the den
                    # matmul single-pass
                    dacc = [dpool.tile([P, FB], bf16, name=f"dacc{h}")
                            for h in range(2)]
                    pt_q = []

                    def consume(jp, pt):
                        if jp == 0:
                            # lazy: ib0's vT/s matmuls borrow these banks
                            # during its produce phase
                            o_tiles.extend(ops.tile([P, FB], f32, name=f"o{cs}")
                                           for cs in range(CT))
                        # accumulator 0 on VectorE, accumulator 1 on the
                        # otherwise-idle GpSimd so neither serial chain gates
                        # pt-tile reuse
                        for h, eng in ((0, nc.vector), (1, nc.gpsimd)):
                            if jp == 0:
                                eng.tensor_copy(dacc[h][:], pt[:, h, :])
                            else:
                                eng.tensor_add(dacc[h][:], dacc[h][:],
                                               pt[:, h, :])
                        for cs in range(CT):
                            nc.tensor.matmul(
                                o_tiles[cs][:],
                                lhsT=vt_sb[:, 2 * jp:2 * jp + 2,
                                           cs * P:(cs + 1) * P],
                                rhs=pt[:],
                                start=(jp == 0), stop=(jp == JP - 1),
                                perf_mode=DR)

                    # ib0 also computes vT and the s bias (both share h_j
                    # stationary tiles with St and must lead the O
                    # consumption), so it produces all 16 pairs first and
                    # consumes after; later ibs pipeline with PIPE=5.
                    pipe = JP if ib == 0 else PIPE
                    for jp in range(JP):
                        pt = ptp.tile([P, 2, FB], f8, name="pt")
                        for h in range(2):
                            jb = 2 * jp + h
                            if ib == 0:
                                # v_j = Wv h_j, transposed layout [j, c];
                                # psum rotates over the idle o0/o1/o2 banks
                                vp = ops.tile([P, C], f32, name=f"o{jb % 3}")
                                for kt in range(0, CT, 2):
                                    nc.tensor.matmul(
                                        vp[:],
                                        lhsT=h_sb[:, kt:kt + 2,
                                                  jb * P:(jb + 1) * P],
                                        rhs=wv_sb[:, kt:kt + 2, :],
                                        start=(kt == 0), stop=(kt == CT - 2),
                                        perf_mode=DR)
                                # split the 32 psum->fp8 casts so VectorE's
                                # FIFO is clear when the consume-phase dacc
                                # chain starts (ScalarE has exp headroom)
                                veng = nc.scalar if jb % 3 == 2 else nc.vector
                                if veng is nc.scalar:
                                    nc.scalar.activation(
                                        vt_sb[:, jb, :], vp[:], AF.Identity,
                                        bias=0.0, scale=1.0)
                                else:
                                    nc.vector.tensor_copy(vt_sb[:, jb, :],
                                                          vp[:])
                                # s[jb] into column jb of the shared sp bank;
                                # ws carries a 128x host scale (fp8 guard)
                                for kt in range(0, CT, 2):
                                    nc.tensor.matmul(
                                        sp[:, jb:jb + 1],
                                        lhsT=h_sb[:, kt:kt + 2,
                                                  jb * P:(jb + 1) * P],
                                        rhs=ws_sb[:, kt:kt + 2, None],
                                        start=(kt == 0), stop=(kt == CT - 2),
                                        perf_mode=DR, skip_group_check=True)
                                nc.vector.tensor_scalar(
                                    out=st_bias[:, jb:jb + 1],
                                    in0=sp[:, jb:jb + 1],
                                    scalar1=1.0 / 128.0, scalar2=None,
                                    op0=ALU.mult)
                            st = sps.tile([P, FB], f32, name="st")
                            for kt in range(0, CT, 2):
                                nc.tensor.matmul(
                                    st[:],
                                    lhsT=h_sb[:, kt:kt + 2,
                                              jb * P:(jb + 1) * P],
                                    rhs=u_sb[:, kt:kt + 2,
                                             ib * FB:(ib + 1) * FB],
                                    start=(kt == 0), stop=(kt == CT - 2),
                                    perf_mode=DR)
                            # wuT carries a 32x host scale; undo it plus the
                            # 1/sqrt(C) attention scale inside the exp, and add
                            # the per-j softmax bias s[j]
                            nc.scalar.activation(pt[:, h, :], st[:], AF.Exp,
                                                 bias=st_bias[:, jb:jb + 1],
                                                 scale=1.0 / (32.0 * math.sqrt(C)))
                        pt_q.append((jp, pt))
                        if ib > 0:
                            if jp == PIPE:
                                # overlap previous block's output projection
                                # with this block's score matmuls
                                final_proj(ib - 1)
                            if ib < NB - 1 and jp in (8, 10, 12, 14):
                                # next block's u, just-in-time, spread out
                                u_ct(ib + 1, (jp - 8) // 2, dps, nc.vector)
                        if jp >= pipe:
                            consume(*pt_q.pop(0))
                    while pt_q:
                        jp_, pt_ = pt_q.pop(0)
                        consume(jp_, pt_)
                        if ib == 0 and NB > 1 and jp_ in (3, 6, 9, 12):
                            # ScalarE is exp-free during ib0's consume drain
                            u_ct(1, (jp_ // 3) - 1, dps, nc.scalar)

                    # all-ones matmul: every psum partition gets sum_j dacc[j,:]
                    denb = dps.tile([P, FB], f32, name="scr")
                    nc.tensor.matmul(denb[:], lhsT=ones128[:], rhs=dacc[0][:],
                                     start=True, stop=False)
                    nc.tensor.matmul(denb[:], lhsT=ones128[:], rhs=dacc[1][:],
                                     start=False, stop=True)
                    rdb = mp.tile([P, FB], f32, name="rdb")
                    nc.vector.reciprocal_approx_fast(rdb[:], denb[:])
                    for cs in range(CT):
                        nc.vector.tensor_mul(
                            attn_sb[:, cs, ib * FB:(ib + 1) * FB],
                            o_tiles[cs][:], rdb[:])
                final_proj(NB - 1)

    nc.compile()
    return nc


def _host_inputs(x, gamma, beta, Wq, bq, Wk, bk, Wv, bv, Wp, bp):
    bf16 = ml_dtypes.bfloat16
    f32 = np.float32
    B = x.shape[0]
    xs = np.asarray(x, f32).reshape(B, C, N)

    def fold(v):
        return np.asarray(v, f32).reshape(CT, P).T.copy()

    f8 = ml_dtypes.float8_e4m3fn

    def wtile(w, scale, dt):
        # [Cout, Cin] -> transposed [Cin, Cout] -> tiled [P, CT, Cout]
        wT = np.asarray(w, f32).T * scale
        return np.ascontiguousarray(
            wT.reshape(CT, P, C).transpose(1, 0, 2)).astype(dt)

    # folded score matrix: softmax_j(q_i.k_j/sqrt(C)) with q=Wq h+bq,
    # k=Wk h+bk equals softmax_j(h_j.(M h_i)/sqrt(C) + s_j) with
    # M = Wk^T Wq and s = (Wk^T bq).h_j/sqrt(C); bk and i-only terms cancel.
    M = np.asarray(Wk, f32).T @ np.asarray(Wq, f32)
    wsv = (np.asarray(Wk, f32).T @ np.asarray(bq, f32)) / math.sqrt(C)
    common = {
        "wuT": wtile(M, 32.0, f8),
        "wvT": wtile(Wv, 16.0, f8),
        "wpT": wtile(Wp, 16.0, f8),
        "ws": (wsv * 128.0).reshape(CT, P).T.copy().astype(f8),
        "gamma": fold(gamma),
        "beta": fold(beta),
    }
    bias_out = (np.asarray(Wp, f32) @ np.asarray(bv, f32)
                + np.asarray(bp, f32)).astype(f32)
    xbs = xs + bias_out[None, :, None]
    g16 = np.zeros((P, P // GS), f32)
    g16[np.arange(P), np.arange(P) // GS] = 1.0 / GS
    gt = np.zeros((P // GS, P), f32)
    gt[np.arange(P) // GS, np.arange(P)] = 1.0
    common["g16"] = g16
    common["gt"] = gt
    return [dict(common, xh=np.ascontiguousarray(xs[b]).astype(bf16),
                 xb=np.ascontiguousarray(xbs[b])) for b in range(B)]


def kernel(x, gamma, beta, Wq, bq, Wk, bk, Wv, bv, Wp, bp, _trace=False):
    from concourse.bass_utils import run_bass_kernel_spmd

    if "nc" not in _CACHE:
        _CACHE["nc"] = _build()
    nc = _CACHE["nc"]
    in_maps = _host_inputs(x, gamma, beta, Wq, bq, Wk, bk, Wv, bv, Wp, bp)
    B = len(in_maps)
    res = run_bass_kernel_spmd(nc, in_maps, core_ids=list(range(B)),
                               trace=_trace)
    out = np.stack([res.results[b]["out"] for b in range(B)])
    out = out.reshape(x.shape).astype(np.float32)
    if _trace:
        _CACHE["last_results"] = res
    return out


# revision 28
# speedup vs baseline: 1.0502x; 1.0009x over previous
"""Trainium2 Bass kernel for an 8-batch AttentionBlock (GroupNorm + single-head
self-attention over 64x64 spatial + residual), data-parallel over batch on 8
NeuronCores (one batch element per core).

Per-core math (x: [512, 4096]):
  h   = groupnorm(x) * gamma + beta       (32 groups of 16 ch; h stored fp8)
  u   = (Wk^T Wq) h                       (folded q/k: one fp8 DoubleRow proj)
  s_j = (Wk^T bq).h_j / sqrt(C)           (per-j softmax bias; bk etc. cancel)
  vT  = (Wv h)^T                          (fp8, layout [j, c], direct - no
                                           transposes anywhere in the kernel)
  St  = h^T u          [j, i] blocks, fp8 DoubleRow matmuls into PSUM f32
  Pt  = exp(St*scale + s_j)  fp8 SBUF (ScalarE runs Exp only - no LUT swaps)
  dacc= sum_jb Pt      bf16 SBUF [128, i]  (VectorE + GpSimd split accumulation)
  denb= ones128^T dacc PSUM (all-ones matmul = column-sum broadcast)
  attn= O * recip(denb)  fp8 at 16x true scale (O = vT^T Pt in PSUM; vT
                         carries a 16x host prescale that is NOT divided out)
  y   = (16 Wp) attn  fp8 DoubleRow; out = y/256 + xb fused on VectorE, where
        xb = x + (Wp bv + bp) is host-folded f32, so the residual is exact.

Schedule (the attention St/O fp8-DoubleRow matmuls run at the PE roofline
~216ns per [128,512]xK512 block, so everything else hides behind them):
  phase A: x lands on 2 DMA queues; groupnorm stats split DVE (bn_stats,
           ct0/2) + ScalarE (activation accum, ct1) + GpSimd (tensor_scalar
           accum, ct3); h writes split DVE/ScalarE/GpSimd. h done ~19us.
  phase B: s-bias matmuls batched into one PSUM bank (columns=jb, one copy),
           u[ib0], then the flash loop. ib0 also computes vT (produce-all/
           consume-all, PIPE=16); later ibs pipeline with PIPE=5, computing
           u[ib+1] just-in-time and the previous block's output projection.
"""

import os
import sys

if "/opt/trn_rl_repo" not in sys.path:
    sys.path.insert(0, "/opt/trn_rl_repo")

# recover automatically if a previous run left the NeuronCores wedged
os.environ.setdefault("NEURON_RT_RESET_CORES", "1")

import math

import ml_dtypes
import numpy as np

C = 512
N = 4096
P = 128
CT = C // P      # 4 channel tiles
FB = 512         # free-dim block (i)
NB = N // FB     # 8 i-blocks
JB = N // P      # 32 j-blocks
JP = JB // 2     # 16 j-block pairs (DoubleRow packs 2 k-subtiles)
GS = 16          # channels per group
EPS = 1e-5
PIPE = 5         # jb-pair delay between St/exp emission and den/O consumption

_CACHE = {}


def _build():
    import concourse.tile as tile
    from concourse import bacc, mybir

    f32 = mybir.dt.float32
    bf16 = mybir.dt.bfloat16
    f8 = mybir.dt.float8e4
    AF = mybir.ActivationFunctionType
    ALU = mybir.AluOpType
    DR = mybir.MatmulPerfMode.DoubleRow

    nc = bacc.Bacc("TRN2", target_bir_lowering=False, debug=False, num_devices=8)

    # bf16 copy of x for the groupnorm/stats path (h is fp8 downstream, so
    # bf16 stats are plenty); the exact f32 x only enters via xb (residual).
    xh_d = nc.dram_tensor("xh", [C, N], bf16, kind="ExternalInput").ap()
    # weights arrive pre-tiled as [P, CT, C] so the load is one contiguous DMA.
    # wuT is the folded score matrix (Wk^T Wq, scaled): softmax(q.k) ==
    # softmax(h.(M h) + s[j]) where s[j] = (Wk^T bq).h_j -- bk and the
    # i-only bias terms cancel inside the softmax.
    wu_d = nc.dram_tensor("wuT", [P, CT, C], f8, kind="ExternalInput").ap()
    wv_d = nc.dram_tensor("wvT", [P, CT, C], f8, kind="ExternalInput").ap()
    wp_d = nc.dram_tensor("wpT", [P, CT, C], f8, kind="ExternalInput").ap()
    ws_d = nc.dram_tensor("ws", [P, CT], f8, kind="ExternalInput").ap()
    # xb = x + (Wp bv + bp) per channel, pre-added on host: the entire
    # residual-plus-output-bias term, so the epilogue is one fused op.
    xb_d = nc.dram_tensor("xb", [C, N], f32, kind="ExternalInput").ap()
    gamma_d = nc.dram_tensor("gamma", [P, CT], f32, kind="ExternalInput").ap()
    beta_d = nc.dram_tensor("beta", [P, CT], f32, kind="ExternalInput").ap()
    g16_d = nc.dram_tensor("g16", [P, P // GS], f32, kind="ExternalInput").ap()
    gt_d = nc.dram_tensor("gt", [P // GS, P], f32, kind="ExternalInput").ap()
    out_d = nc.dram_tensor("out", [C, N], f32, kind="ExternalOutput").ap()

    with tile.TileContext(nc) as tc:
        from contextlib import ExitStack

        with ExitStack() as ctx:
            consts = ctx.enter_context(tc.tile_pool(name="consts", bufs=1))
            big = ctx.enter_context(tc.tile_pool(name="big", bufs=1))
            xpool = ctx.enter_context(tc.tile_pool(name="p1", bufs=CT))

            # x feeds the groupnorm critical path - issue its DMAs before
            # anything else lands on the queues (descriptor issue is serial,
            # ~0.6us each). ct0/2 on sync, ct1/3 on scalar: each stats engine
            # gets its tile as early as possible.
            x_tiles = [None] * CT
            for ct in range(CT):
                x_tiles[ct] = xpool.tile([P, N], bf16, name="xt")
            # sync ring: ct0 then ct2 (the DVE bn_stats order); scalar ring:
            # ct1 then ct3's SECOND half first (ScalarE's accum passes need
            # it; DVE picks up ct3's first half last, after ct0/ct2)
            H2 = N // 2
            chunks = {nc.sync: [(0, 0), (0, 1), (2, 0), (2, 1)],
                      nc.scalar: [(1, 0), (1, 1), (3, 1), (3, 0)]}
            for eng, lst in chunks.items():
                for ct, hh in lst:
                    sl = slice(hh * H2, (hh + 1) * H2)
                    eng.dma_start(x_tiles[ct][:, sl],
                                  xh_d[ct * P:(ct + 1) * P, sl])

            def load_w(dram, nm, dt):  # noqa: E306
                t = consts.tile([P, CT, C], dt, name=nm)
                nc.sync.dma_start(t[:], dram)
                return t

            wu_sb = load_w(wu_d, "wu_sb", f8)
            wv_sb = load_w(wv_d, "wv_sb", f8)
            wp_sb = load_w(wp_d, "wp_sb", f8)

            def load_small(dram, shape, nm, dt=f32):
                # gpsimd queue: don't let these tiny loads (needed early by
                # the groupnorm chain) queue behind the big weight DMAs
                t = consts.tile(shape, dt, name=nm)
                nc.gpsimd.dma_start(t[:], dram)
                return t

            ws_sb = load_small(ws_d, [P, CT], "ws_sb", f8)
            gamma_sb = load_small(gamma_d, [P, CT], "gamma_sb")
            beta_sb = load_small(beta_d, [P, CT], "beta_sb")
            g16_sb = load_small(g16_d, [P, P // GS], "g16_sb")
            gt_sb = load_small(gt_d, [P // GS, P], "gt_sb")

            # all-ones weight for the denominator column-sum matmul. vT keeps
            # its 16x fp8 prescale un-divided (attn is stored at 16x true
            # scale, in fp8's sweet spot); wp carries another 16x, and the
            # epilogue divides the combined 256x back out.
            ones128 = consts.tile([P, P], bf16, name="ones128")
            nc.vector.memset(ones128[:], 1.0)
            eps_sb = consts.tile([P // GS, 1], f32, name="eps_sb")
            nc.vector.memset(eps_sb[:], EPS)

            u_sb = big.tile([P, CT, N], f8, name="u")
            vt_sb = big.tile([P, JB, C], f8, name="vt")
            h_sb = big.tile([P, CT, N], f8, name="h")
            # per-j additive softmax bias s[j] (see wuT comment), f32
            st_bias = big.tile([P, JB], f32, name="st_bias")
            attn_sb = big.tile([P, CT, N], f8, name="attn")
            # full-size dummy outs for the accum-based stats paths (one per
            # engine so the passes don't serialize on a shared scratch)
            scr_a = big.tile([P, N], bf16, name="scr_a")
            scr_b = big.tile([P, N], bf16, name="scr_b")

            # shared matmul psum pool (u blocks + St blocks + v blocks)
            sps = ctx.enter_context(tc.tile_pool(name="sps", bufs=2, space="PSUM"))

            # ---------------- phase A: groupnorm -> h ----------------------
            with tc.tile_pool(name="p1s", bufs=2) as p1s, \
                 tc.tile_pool(name="gnps", bufs=1, space="PSUM") as gnps:
                # dummy matmuls warm the PE HAM clock-gate (~3.4us of
                # activity -> 2.4GHz) while the stats chains run; PE would
                # otherwise start the attention matmuls cold
                warm = gnps.tile([P, P], f32, name="warm")

                def warmup(k):
                    for _ in range(k):
                        nc.tensor.matmul(warm[:], lhsT=ones128[:],
                                         rhs=ones128[:], start=True, stop=True)

                warmup(16)
                # stats split: DVE bn_stats for ct0, ct2, and the first half
                # of ct3; ScalarE Identity/Square accum passes for ct1 and the
                # second half of ct3 (I,I,S,S order: one act-table swap).
                # ms_all cols [2ct, 2ct+1] = [mean, E[x^2]] per channel, f32.
                ms_all = p1s.tile([P, 2 * CT], f32, name="ms_all")
                mv_all = p1s.tile([P, 4], f32, name="mv_all")
                mv3 = p1s.tile([P, 2], f32, name="mv3")
                acc3 = p1s.tile([P, 2], f32, name="acc3")
                s2t = p1s.tile([P, 2], f32, name="s2t")
                Nh = N // 2
                for ct in (0, 2):
                    x_t = x_tiles[ct]
                    stats = p1s.tile([P, 8, 6], f32, name="stats")
                    for sg in range(8):
                        nc.vector.bn_stats(
                            stats[:, sg, :], x_t[:, sg * 512:(sg + 1) * 512])
                    nc.vector.bn_aggr(mv_all[:, ct:ct + 2], stats[:])
                    warmup(6)
                stats3 = p1s.tile([P, 4, 6], f32, name="stats3")
                for sg in range(4):
                    nc.vector.bn_stats(
                        stats3[:, sg, :], x_tiles[3][:, sg * 512:(sg + 1) * 512])
                nc.vector.bn_aggr(mv3[:], stats3[:])
                # ScalarE: means first, then squares (batching the act funcs)
                nc.scalar.activation(scr_a[:], x_tiles[1][:], AF.Identity,
                                     bias=0.0, scale=1.0 / N,
                                     accum_out=ms_all[:, 2:3])
                nc.scalar.activation(scr_b[:, :Nh], x_tiles[3][:, Nh:],
                                     AF.Identity, bias=0.0, scale=1.0 / Nh,
                                     accum_out=acc3[:, 0:1])
                nc.scalar.activation(scr_a[:], x_tiles[1][:], AF.Square,
                                     bias=0.0, scale=1.0,
                                     accum_out=s2t[:, 0:1])
                nc.scalar.activation(scr_b[:, :Nh], x_tiles[3][:, Nh:],
                                     AF.Square, bias=0.0, scale=1.0,
                                     accum_out=s2t[:, 1:2])
                warmup(8)
                # assemble ms_all: bn cts (strided), ct1 E[x^2], ct3 combine
                gmt = p1s.tile([P, 2], f32, name="gmt")
                nc.gpsimd.tensor_copy(ms_all[:, 0::4], mv_all[:, 0::2])
                nc.gpsimd.tensor_mul(gmt[:], mv_all[:, 0::2], mv_all[:, 0::2])
                nc.gpsimd.tensor_add(ms_all[:, 1::4], mv_all[:, 1::2], gmt[:])
                nc.gpsimd.tensor_scalar(out=ms_all[:, 3:4], in0=s2t[:, 0:1],
                                        scalar1=1.0 / N, scalar2=None,
                                        op0=ALU.mult)
                # ct3: half a from bn (mean_a, var_a), half b from accums
                e2a = p1s.tile([P, 1], f32, name="e2a")
                nc.vector.tensor_mul(e2a[:], mv3[:, 0:1], mv3[:, 0:1])
                nc.vector.tensor_add(e2a[:], e2a[:], mv3[:, 1:2])
                nc.vector.tensor_add(ms_all[:, 6:7], mv3[:, 0:1], acc3[:, 0:1])
                nc.vector.tensor_scalar(out=ms_all[:, 6:7], in0=ms_all[:, 6:7],
                                        scalar1=0.5, scalar2=None, op0=ALU.mult)
                nc.vector.tensor_scalar(out=e2a[:], in0=e2a[:], scalar1=0.5,
                                        scalar2=None, op0=ALU.mult)
                nc.vector.tensor_scalar(out=ms_all[:, 7:8], in0=s2t[:, 1:2],
                                        scalar1=0.5 / Nh, scalar2=None,
                                        op0=ALU.mult)
                nc.vector.tensor_add(ms_all[:, 7:8], ms_all[:, 7:8], e2a[:])

                # one batched group-norm chain for all 4 cts ([8, 2] per ct)
                gps = gnps.tile([P // GS, 2 * CT], f32, name="gps")
                nc.tensor.matmul(gps[:], lhsT=g16_sb[:], rhs=ms_all[:],
                                 start=True, stop=True)
                gsb = p1s.tile([P // GS, 2 * CT], f32, name="gsb")
                nc.vector.tensor_copy(gsb[:], gps[:])
                gm2 = p1s.tile([P // GS, CT], f32, name="gm2")
                nc.gpsimd.tensor_mul(gm2[:], gsb[:, 0::2], gsb[:, 0::2])
                nc.gpsimd.tensor_sub(gsb[:, 1::2], gsb[:, 1::2], gm2[:])
                # var cols -> 1/sqrt(var + eps)
                nc.scalar.activation(gsb[:, 1::2], gsb[:, 1::2], AF.Sqrt,
                                     bias=eps_sb[:], scale=1.0)
                # dummy exp: swap the act table to the exp set now, while the
                # h writes / u copies run, so the first real exp doesn't pay
                # the ~1.3us LUT load (identity lives in every set)
                nc.scalar.activation(s2t[:, 0:1], s2t[:, 0:1], AF.Exp,
                                     bias=0.0, scale=0.0)
                nc.vector.reciprocal_approx_fast(gsb[:, 1::2], gsb[:, 1::2])
                # broadcast group (mean, rstd) back to 128 channels
                cps = gnps.tile([P, 2 * CT], f32, name="cps")
                nc.tensor.matmul(cps[:], lhsT=gt_sb[:], rhs=gsb[:],
                                 start=True, stop=True)
                scale_all = p1s.tile([P, CT], f32, name="scale_all")
                nc.vector.tensor_mul(scale_all[:], cps[:, 1::2], gamma_sb[:])
                nb1 = p1s.tile([P, CT], f32, name="nb1")
                nc.vector.tensor_mul(nb1[:], cps[:, 0::2], scale_all[:])
                nbias_all = p1s.tile([P, CT], f32, name="nbias_all")
                nc.vector.tensor_sub(nbias_all[:], beta_sb[:], nb1[:])
                warmup(6)
                # h writes, spread across DVE / GpSimd / ScalarE (DVE is
                # fastest per chunk, so it takes 4 of the 8)
                h_engs = (nc.vector, nc.vector, nc.scalar, nc.gpsimd,
                          nc.vector, nc.scalar, nc.gpsimd, nc.vector)
                for ct in range(CT):
                    x_t = x_tiles[ct]
                    for hh in range(2):
                        sl = slice(hh * (N // 2), (hh + 1) * (N // 2))
                        eng = h_engs[2 * ct + hh]
                        if eng is nc.scalar:
                            nc.scalar.activation(
                                h_sb[:, ct, sl], x_t[:, sl], AF.Identity,
                                bias=nbias_all[:, ct:ct + 1],
                                scale=scale_all[:, ct:ct + 1])
                        else:
                            eng.tensor_scalar(
                                out=h_sb[:, ct, sl], in0=x_t[:, sl],
                                scalar1=scale_all[:, ct:ct + 1],
                                scalar2=nbias_all[:, ct:ct + 1],
                                op0=ALU.mult, op1=ALU.add)
                    warmup(4)

            # ---------------- phase B: attention -----------------------------
            with tc.tile_pool(name="ptpool", bufs=JP + 2) as ptp, \
                 tc.tile_pool(name="ops", bufs=1, space="PSUM") as ops, \
                 tc.tile_pool(name="dps", bufs=2, space="PSUM") as dps, \
                 tc.tile_pool(name="dpool", bufs=2) as dpool, \
                 tc.tile_pool(name="mpool", bufs=2) as mp, \
                 tc.tile_pool(name="xrpool", bufs=5) as xrp, \
                 tc.tile_pool(name="outpool", bufs=3) as outp:

                # s-bias scratch: one PSUM bank (shares the o3 bank; ib0's
                # o_tiles are allocated only at first consume, after all
                # s matmuls have been copied out), one column per j-block.
                # Full-bank shape so every o3 allocation is the same size.
                sp = ops.tile([P, FB], f32, name="o3")

                def u_ct(ib, ct, pool, copy_eng):
                    # one [128, FB] chunk of u[:, :, ib-block]: 2 DoubleRow
                    # matmuls + a PSUM copy. JIT chunks borrow the dps 'scr'
                    # banks (idle between final-proj bursts) so the St psum
                    # rotation in sps never waits on a u copy.
                    qp = pool.tile([P, FB], f32,
                                   name="st" if pool is sps else "scr")
                    for kt in range(0, CT, 2):
                        nc.tensor.matmul(
                            qp[:],
                            lhsT=wu_sb[:, kt:kt + 2, ct * P:(ct + 1) * P],
                            rhs=h_sb[:, kt:kt + 2, ib * FB:(ib + 1) * FB],
                            start=(kt == 0), stop=(kt == CT - 2),
                            perf_mode=DR)
                    if copy_eng is nc.scalar:
                        nc.scalar.activation(
                            u_sb[:, ct, ib * FB:(ib + 1) * FB], qp[:],
                            AF.Identity, bias=0.0, scale=1.0)
                    else:
                        copy_eng.tensor_copy(
                            u_sb[:, ct, ib * FB:(ib + 1) * FB], qp[:])

                for ct in range(CT):
                    u_ct(0, ct, sps, nc.vector)

                def final_proj(ib):
                    xrs = []
                    for ct in range(CT):
                        xr = xrp.tile([P, FB], f32, name="xr")
                        nc.sync.dma_start(
                            xr[:], xb_d[ct * P:(ct + 1) * P, ib * FB:(ib + 1) * FB])
                        xrs.append(xr)
                    for ct in range(CT):
                        yp = dps.tile([P, FB], f32, name="scr")
                        for kt in range(0, CT, 2):
                            nc.tensor.matmul(
                                yp[:],
                                lhsT=wp_sb[:, kt:kt + 2, ct * P:(ct + 1) * P],
                                rhs=attn_sb[:, kt:kt + 2, ib * FB:(ib + 1) * FB],
                                start=(kt == 0), stop=(kt == CT - 2),
                                perf_mode=DR)
                        ot = outp.tile([P, FB], f32, name="ot")
                        # y/256 + xb fused: undoes the 16x on vT and 16x on wp
                        nc.vector.scalar_tensor_tensor(
                            out=ot[:], in0=yp[:], scalar=1.0 / 256.0,
                            in1=xrs[ct][:], op0=ALU.mult, op1=ALU.add)
                        nc.sync.dma_start(
                            out_d[ct * P:(ct + 1) * P, ib * FB:(ib + 1) * FB],
                            ot[:])

                for ib in range(NB):
                    o_tiles = []
                    # two independent denominator accumulators halve the
                    # serial DVE chain; bf16 is plenty (errors average out
                    # 1/sqrt(128) in the column-sum matmul) and keeps the den
                    # matmul single-pass
                    dacc = [dpool.tile([P, FB], bf16, name=f"dacc{h}")
                            for h in range(2)]
                    pt_q = []

                    def consume(jp, pt):
                        if jp == 0:
                            # lazy: ib0's vT/s matmuls borrow these banks
                            # during its produce phase
                            o_tiles.extend(ops.tile([P, FB], f32, name=f"o{cs}")
                                           for cs in range(CT))
                        # accumulator 0 on VectorE, accumulator 1 on the
                        # otherwise-idle GpSimd so neither serial chain gates
                        # pt-tile reuse
                        for h, eng in ((0, nc.vector), (1, nc.gpsimd)):
                            if jp == 0:
                                eng.tensor_copy(dacc[h][:], pt[:, h, :])
                            else:
                                eng.tensor_add(dacc[h][:], dacc[h][:],
                                               pt[:, h, :])
                        for cs in range(CT):
                            nc.tensor.matmul(
                                o_tiles[cs][:],
                                lhsT=vt_sb[:, 2 * jp:2 * jp + 2,
                                           cs * P:(cs + 1) * P],
                                rhs=pt[:],
                                start=(jp == 0), stop=(jp == JP - 1),
                                perf_mode=DR)

                    # ib0 also computes vT and the s bias (both share h_j
                    # stationary tiles with St and must lead the O
                    # consumption), so it produces all 16 pairs first and
                    # consumes after; later ibs pipeline with PIPE=5.
                    pipe = JP if ib == 0 else PIPE
                    for jp in range(JP):
                        pt = ptp.tile([P, 2, FB], f8, name="pt")
                        for h in range(2):
                            jb = 2 * jp + h
                            if ib == 0:
                                # s[jb] into column jb of the shared sp bank
                                # FIRST (exp(jb) needs it; the tiny copy must
                                # not queue behind a vt cast on VectorE).
                                # ws carries a 128x host scale (fp8 guard)
                                for kt in range(0, CT, 2):
                                    nc.tensor.matmul(
                                        sp[:, jb:jb + 1],
                                        lhsT=h_sb[:, kt:kt + 2,
                                                  jb * P:(jb + 1) * P],
                                        rhs=ws_sb[:, kt:kt + 2, None],
                                        start=(kt == 0), stop=(kt == CT - 2),
                                        perf_mode=DR, skip_group_check=True)
                                nc.vector.tensor_scalar(
                                    out=st_bias[:, jb:jb + 1],
                                    in0=sp[:, jb:jb + 1],
                                    scalar1=1.0 / 128.0, scalar2=None,
                                    op0=ALU.mult)
                            st = sps.tile([P, FB], f32, name="st")
                            for kt in range(0, CT, 2):
                                nc.tensor.matmul(
                                    st[:],
                                    lhsT=h_sb[:, kt:kt + 2,
                                              jb * P:(jb + 1) * P],
                                    rhs=u_sb[:, kt:kt + 2,
                                             ib * FB:(ib + 1) * FB],
                                    start=(kt == 0), stop=(kt == CT - 2),
                                    perf_mode=DR)
                            # wuT carries a 32x host scale; undo it plus the
                            # 1/sqrt(C) attention scale inside the exp, and add
                            # the per-j softmax bias s[j]
                            nc.scalar.activation(pt[:, h, :], st[:], AF.Exp,
                                                 bias=st_bias[:, jb:jb + 1],
                                                 scale=1.0 / (32.0 * math.sqrt(C)))
                            if ib == 0:
                                # v_j = Wv h_j, transposed layout [j, c];
                                # psum rotates over the idle o0/o1/o2 banks
                                vp = ops.tile([P, C], f32, name=f"o{jb % 3}")
                                for kt in range(0, CT, 2):
                                    nc.tensor.matmul(
                                        vp[:],
                                        lhsT=h_sb[:, kt:kt + 2,
                                                  jb * P:(jb + 1) * P],
                                        rhs=wv_sb[:, kt:kt + 2, :],
                                        start=(kt == 0), stop=(kt == CT - 2),
                                        perf_mode=DR)
                                # split the 32 psum->fp8 casts so VectorE's
                                # FIFO is clear when the consume-phase dacc
                                # chain starts (ScalarE has exp headroom)
                                if jb % 3 == 2:
                                    nc.scalar.activation(
                                        vt_sb[:, jb, :], vp[:], AF.Identity,
                                        bias=0.0, scale=1.0)
                                else:
                                    nc.vector.tensor_copy(vt_sb[:, jb, :],
                                                          vp[:])
                        pt_q.append((jp, pt))
                        if ib > 0:
                            if jp == PIPE:
                                # overlap previous block's output projection
                                # with this block's score matmuls
                                final_proj(ib - 1)
                            if ib < NB - 1 and jp in (8, 10, 12, 14):
                                # next block's u, just-in-time, spread out
                                u_ct(ib + 1, (jp - 8) // 2, dps, nc.vector)
                        if jp >= pipe:
                            consume(*pt_q.pop(0))
                    while pt_q:
                        jp_, pt_ = pt_q.pop(0)
                        consume(jp_, pt_)
                        if ib == 0 and NB > 1 and jp_ in (3, 6, 9, 12):
                            # ScalarE is exp-free during ib0's consume drain
                            u_ct(1, (jp_ // 3) - 1, dps, nc.scalar)

                    # all-ones matmul: every psum partition gets sum_j dacc[j,:]
                    denb = dps.tile([P, FB], f32, name="scr")
                    nc.tensor.matmul(denb[:], lhsT=ones128[:], rhs=dacc[0][:],
                                     start=True, stop=False)
                    nc.tensor.matmul(denb[:], lhsT=ones128[:], rhs=dacc[1][:],
                                     start=False, stop=True)
                    rdb = mp.tile([P, FB], f32, name="rdb")
                    nc.vector.reciprocal_approx_fast(rdb[:], denb[:])
                    for cs in range(CT):
                        nc.vector.tensor_mul(
                            attn_sb[:, cs, ib * FB:(ib + 1) * FB],
                            o_tiles[cs][:], rdb[:])
                final_proj(NB - 1)

    nc.compile()
    return nc


def _host_inputs(x, gamma, beta, Wq, bq, Wk, bk, Wv, bv, Wp, bp):
    bf16 = ml_dtypes.bfloat16
    f32 = np.float32
    B = x.shape[0]
    xs = np.asarray(x, f32).reshape(B, C, N)

    def fold(v):
        return np.asarray(v, f32).reshape(CT, P).T.copy()

    f8 = ml_dtypes.float8_e4m3fn

    def wtile(w, scale, dt):
        # [Cout, Cin] -> transposed [Cin, Cout] -> tiled [P, CT, Cout]
        wT = np.asarray(w, f32).T * scale
        return np.ascontiguousarray(
            wT.reshape(CT, P, C).transpose(1, 0, 2)).astype(dt)

    # folded score matrix: softmax_j(q_i.k_j/sqrt(C)) with q=Wq h+bq,
    # k=Wk h+bk equals softmax_j(h_j.(M h_i)/sqrt(C) + s_j) with
    # M = Wk^T Wq and s = (Wk^T bq).h_j/sqrt(C); bk and i-only terms cancel.
    M = np.asarray(Wk, f32).T @ np.asarray(Wq, f32)
    wsv = (np.asarray(Wk, f32).T @ np.asarray(bq, f32)) / math.sqrt(C)
    common = {
        "wuT": wtile(M, 32.0, f8),
        "wvT": wtile(Wv, 16.0, f8),
        "wpT": wtile(Wp, 16.0, f8),
        "ws": (wsv * 128.0).reshape(CT, P).T.copy().astype(f8),
        "gamma": fold(gamma),
        "beta": fold(beta),
    }
    bias_out = (np.asarray(Wp, f32) @ np.asarray(bv, f32)
                + np.asarray(bp, f32)).astype(f32)
    xbs = xs + bias_out[None, :, None]
    g16 = np.zeros((P, P // GS), f32)
    g16[np.arange(P), np.arange(P) // GS] = 1.0 / GS
    gt = np.zeros((P // GS, P), f32)
    gt[np.arange(P) // GS, np.arange(P)] = 1.0
    common["g16"] = g16
    common["gt"] = gt
    return [dict(common, xh=np.ascontiguousarray(xs[b]).astype(bf16),
                 xb=np.ascontiguousarray(xbs[b])) for b in range(B)]


def kernel(x, gamma, beta, Wq, bq, Wk, bk, Wv, bv, Wp, bp, _trace=False):
    from concourse.bass_utils import run_bass_kernel_spmd

    if "nc" not in _CACHE:
        _CACHE["nc"] = _build()
    nc = _CACHE["nc"]
    in_maps = _host_inputs(x, gamma, beta, Wq, bq, Wk, bk, Wv, bv, Wp, bp)
    B = len(in_maps)
    res = run_bass_kernel_spmd(nc, in_maps, core_ids=list(range(B)),
                               trace=_trace)
    out = np.stack([res.results[b]["out"] for b in range(B)])
    out = out.reshape(x.shape).astype(np.float32)
    if _trace:
        _CACHE["last_results"] = res
    return out


# revision 29
# speedup vs baseline: 1.0706x; 1.0194x over previous
"""Trainium2 Bass kernel for an 8-batch AttentionBlock (GroupNorm + single-head
self-attention over 64x64 spatial + residual), data-parallel over batch on 8
NeuronCores (one batch element per core).

Per-core math (x: [512, 4096]):
  h   = groupnorm(x) * gamma + beta       (32 groups of 16 ch; h stored fp8)
  u   = (Wk^T Wq) h                       (folded q/k: one fp8 DoubleRow proj)
  s_j = (Wk^T bq).h_j / sqrt(C)           (per-j softmax bias; bk etc. cancel)
  vT  = (Wv h)^T                          (fp8, layout [j, c], direct - no
                                           transposes anywhere in the kernel)
  St  = h^T u          [j, i] blocks, fp8 DoubleRow matmuls into PSUM f32
  Pt  = exp(St*scale + s_j)  fp8 SBUF (ScalarE runs Exp only - no LUT swaps)
  dacc= sum_jb Pt      bf16 SBUF [128, i]  (VectorE + GpSimd split accumulation)
  denb= ones128^T dacc PSUM (all-ones matmul = column-sum broadcast)
  attn= O * recip(denb)  fp8 at 16x true scale (O = vT^T Pt in PSUM; vT
                         carries a 16x host prescale that is NOT divided out)
  y   = (16 Wp) attn  fp8 DoubleRow; out = y/256 + xb fused on VectorE, where
        xb = x + (Wp bv + bp) is host-folded f32, so the residual is exact.

Schedule (the attention St/O fp8-DoubleRow matmuls run at the PE roofline
~216ns per [128,512]xK512 block, so everything else hides behind them):
  phase A: x lands on 2 DMA queues; groupnorm stats split DVE (bn_stats,
           ct0/2) + ScalarE (activation accum, ct1) + GpSimd (tensor_scalar
           accum, ct3); h writes split DVE/ScalarE/GpSimd. h done ~19us.
  phase B: s-bias matmuls batched into one PSUM bank (columns=jb, one copy),
           u[ib0], then the flash loop. ib0 also computes vT (produce-all/
           consume-all, PIPE=16); later ibs pipeline with PIPE=5, computing
           u[ib+1] just-in-time and the previous block's output projection.
"""

import os
import sys

if "/opt/trn_rl_repo" not in sys.path:
    sys.path.insert(0, "/opt/trn_rl_repo")

# recover automatically if a previous run left the NeuronCores wedged
os.environ.setdefault("NEURON_RT_RESET_CORES", "1")

import math

import ml_dtypes
import numpy as np

C = 512
N = 4096
P = 128
CT = C // P      # 4 channel tiles
FB = 512         # free-dim block (i)
NB = N // FB     # 8 i-blocks
JB = N // P      # 32 j-blocks
JP = JB // 2     # 16 j-block pairs (DoubleRow packs 2 k-subtiles)
GS = 16          # channels per group
EPS = 1e-5
PIPE = 5         # jb-pair delay between St/exp emission and den/O consumption

_CACHE = {}


def _build():
    import concourse.tile as tile
    from concourse import bacc, mybir

    f32 = mybir.dt.float32
    bf16 = mybir.dt.bfloat16
    f8 = mybir.dt.float8e4
    AF = mybir.ActivationFunctionType
    ALU = mybir.AluOpType
    DR = mybir.MatmulPerfMode.DoubleRow

    nc = bacc.Bacc("TRN2", target_bir_lowering=False, debug=False, num_devices=8)

    # fp8 copy of x for the groupnorm/stats path (h is quantized to fp8
    # downstream anyway, and every engine reads fp8 at full rate, so this
    # halves the critical x DMA); the exact f32 x only enters via xb.
    xh_d = nc.dram_tensor("xh", [C, N], f8, kind="ExternalInput").ap()
    # weights arrive pre-tiled as [P, CT, C] so the load is one contiguous DMA.
    # wuT is the folded score matrix (Wk^T Wq, scaled): softmax(q.k) ==
    # softmax(h.(M h) + s[j]) where s[j] = (Wk^T bq).h_j -- bk and the
    # i-only bias terms cancel inside the softmax.
    wu_d = nc.dram_tensor("wuT", [P, CT, C], f8, kind="ExternalInput").ap()
    wv_d = nc.dram_tensor("wvT", [P, CT, C], f8, kind="ExternalInput").ap()
    wp_d = nc.dram_tensor("wpT", [P, CT, C], f8, kind="ExternalInput").ap()
    ws_d = nc.dram_tensor("ws", [P, CT], f8, kind="ExternalInput").ap()
    # xb = x + (Wp bv + bp) per channel, pre-added on host: the entire
    # residual-plus-output-bias term, so the epilogue is one fused op.
    xb_d = nc.dram_tensor("xb", [C, N], f32, kind="ExternalInput").ap()
    gamma_d = nc.dram_tensor("gamma", [P, CT], f32, kind="ExternalInput").ap()
    beta_d = nc.dram_tensor("beta", [P, CT], f32, kind="ExternalInput").ap()
    g16_d = nc.dram_tensor("g16", [P, P // GS], f32, kind="ExternalInput").ap()
    gt_d = nc.dram_tensor("gt", [P // GS, P], f32, kind="ExternalInput").ap()
    out_d = nc.dram_tensor("out", [C, N], f32, kind="ExternalOutput").ap()

    with tile.TileContext(nc) as tc:
        from contextlib import ExitStack

        with ExitStack() as ctx:
            consts = ctx.enter_context(tc.tile_pool(name="consts", bufs=1))
            big = ctx.enter_context(tc.tile_pool(name="big", bufs=1))
            xpool = ctx.enter_context(tc.tile_pool(name="p1", bufs=CT))

            # x feeds the groupnorm critical path - issue its DMAs before
            # anything else lands on the queues (descriptor issue is serial,
            # ~0.6us each). ct0/2 on sync, ct1/3 on scalar: each stats engine
            # gets its tile as early as possible.
            x_tiles = [None] * CT
            for ct in range(CT):
                x_tiles[ct] = xpool.tile([P, N], f8, name="xt")
            # sync ring: ct0 then ct2 (the DVE bn_stats order); scalar ring:
            # ct1 then ct3's SECOND half first (ScalarE's accum passes need
            # it; DVE picks up ct3's first half last, after ct0/ct2)
            H2 = N // 2
            chunks = {nc.sync: [(0, 0), (0, 1), (2, 0), (2, 1)],
                      nc.scalar: [(1, 0), (1, 1), (3, 1), (3, 0)]}
            for eng, lst in chunks.items():
                for ct, hh in lst:
                    sl = slice(hh * H2, (hh + 1) * H2)
                    eng.dma_start(x_tiles[ct][:, sl],
                                  xh_d[ct * P:(ct + 1) * P, sl])

            def load_w(dram, nm, dt):  # noqa: E306
                t = consts.tile([P, CT, C], dt, name=nm)
                nc.sync.dma_start(t[:], dram)
                return t

            wu_sb = load_w(wu_d, "wu_sb", f8)
            wv_sb = load_w(wv_d, "wv_sb", f8)
            wp_sb = load_w(wp_d, "wp_sb", f8)

            def load_small(dram, shape, nm, dt=f32):
                # gpsimd queue: don't let these tiny loads (needed early by
                # the groupnorm chain) queue behind the big weight DMAs
                t = consts.tile(shape, dt, name=nm)
                nc.gpsimd.dma_start(t[:], dram)
                return t

            ws_sb = load_small(ws_d, [P, CT], "ws_sb", f8)
            gamma_sb = load_small(gamma_d, [P, CT], "gamma_sb")
            beta_sb = load_small(beta_d, [P, CT], "beta_sb")
            g16_sb = load_small(g16_d, [P, P // GS], "g16_sb")
            gt_sb = load_small(gt_d, [P // GS, P], "gt_sb")

            # all-ones weight for the denominator column-sum matmul. vT keeps
            # its 16x fp8 prescale un-divided (attn is stored at 16x true
            # scale, in fp8's sweet spot); wp carries another 16x, and the
            # epilogue divides the combined 256x back out.
            ones128 = consts.tile([P, P], bf16, name="ones128")
            nc.vector.memset(ones128[:], 1.0)
            eps_sb = consts.tile([P // GS, 1], f32, name="eps_sb")
            nc.vector.memset(eps_sb[:], EPS)

            u_sb = big.tile([P, CT, N], f8, name="u")
            vt_sb = big.tile([P, JB, C], f8, name="vt")
            h_sb = big.tile([P, CT, N], f8, name="h")
            # per-j additive softmax bias s[j] (see wuT comment), f32
            st_bias = big.tile([P, JB], f32, name="st_bias")
            attn_sb = big.tile([P, CT, N], f8, name="attn")
            # full-size dummy outs for the accum-based stats paths (one per
            # engine so the passes don't serialize on a shared scratch)
            scr_a = big.tile([P, N], bf16, name="scr_a")
            scr_b = big.tile([P, N], bf16, name="scr_b")

            # shared matmul psum pool (u blocks + St blocks + v blocks)
            sps = ctx.enter_context(tc.tile_pool(name="sps", bufs=2, space="PSUM"))

            # ---------------- phase A: groupnorm -> h ----------------------
            with tc.tile_pool(name="p1s", bufs=2) as p1s, \
                 tc.tile_pool(name="gnps", bufs=1, space="PSUM") as gnps:
                # dummy matmuls warm the PE HAM clock-gate (~3.4us of
                # activity -> 2.4GHz) while the stats chains run; PE would
                # otherwise start the attention matmuls cold
                warm = gnps.tile([P, P], f32, name="warm")

                def warmup(k):
                    for _ in range(k):
                        nc.tensor.matmul(warm[:], lhsT=ones128[:],
                                         rhs=ones128[:], start=True, stop=True)

                warmup(16)
                # stats split: DVE bn_stats for ct0, ct2, and the first half
                # of ct3; ScalarE Identity/Square accum passes for ct1 and the
                # second half of ct3 (I,I,S,S order: one act-table swap).
                # ms_all cols [2ct, 2ct+1] = [mean, E[x^2]] per channel, f32.
                ms_all = p1s.tile([P, 2 * CT], f32, name="ms_all")
                mv_all = p1s.tile([P, 4], f32, name="mv_all")
                mv3 = p1s.tile([P, 2], f32, name="mv3")
                acc3 = p1s.tile([P, 2], f32, name="acc3")
                s2t = p1s.tile([P, 2], f32, name="s2t")
                Nh = N // 2
                for ct in (0, 2):
                    x_t = x_tiles[ct]
                    stats = p1s.tile([P, 8, 6], f32, name="stats")
                    for sg in range(8):
                        nc.vector.bn_stats(
                            stats[:, sg, :], x_t[:, sg * 512:(sg + 1) * 512])
                    nc.vector.bn_aggr(mv_all[:, ct:ct + 2], stats[:])
                    warmup(6)
                stats3 = p1s.tile([P, 4, 6], f32, name="stats3")
                for sg in range(4):
                    nc.vector.bn_stats(
                        stats3[:, sg, :], x_tiles[3][:, sg * 512:(sg + 1) * 512])
                nc.vector.bn_aggr(mv3[:], stats3[:])
                # ScalarE: means first, then squares (batching the act funcs)
                nc.scalar.activation(scr_a[:], x_tiles[1][:], AF.Identity,
                                     bias=0.0, scale=1.0 / N,
                                     accum_out=ms_all[:, 2:3])
                nc.scalar.activation(scr_b[:, :Nh], x_tiles[3][:, Nh:],
                                     AF.Identity, bias=0.0, scale=1.0 / Nh,
                                     accum_out=acc3[:, 0:1])
                nc.scalar.activation(scr_a[:], x_tiles[1][:], AF.Square,
                                     bias=0.0, scale=1.0,
                                     accum_out=s2t[:, 0:1])
                nc.scalar.activation(scr_b[:, :Nh], x_tiles[3][:, Nh:],
                                     AF.Square, bias=0.0, scale=1.0,
                                     accum_out=s2t[:, 1:2])
                warmup(8)
                # assemble ms_all: bn cts (strided), ct1 E[x^2], ct3 combine
                gmt = p1s.tile([P, 2], f32, name="gmt")
                nc.gpsimd.tensor_copy(ms_all[:, 0::4], mv_all[:, 0::2])
                nc.gpsimd.tensor_mul(gmt[:], mv_all[:, 0::2], mv_all[:, 0::2])
                nc.gpsimd.tensor_add(ms_all[:, 1::4], mv_all[:, 1::2], gmt[:])
                nc.gpsimd.tensor_scalar(out=ms_all[:, 3:4], in0=s2t[:, 0:1],
                                        scalar1=1.0 / N, scalar2=None,
                                        op0=ALU.mult)
                # ct3: half a from bn (mean_a, var_a), half b from accums
                e2a = p1s.tile([P, 1], f32, name="e2a")
                nc.vector.tensor_mul(e2a[:], mv3[:, 0:1], mv3[:, 0:1])
                nc.vector.tensor_add(e2a[:], e2a[:], mv3[:, 1:2])
                nc.vector.tensor_add(ms_all[:, 6:7], mv3[:, 0:1], acc3[:, 0:1])
                nc.vector.tensor_scalar(out=ms_all[:, 6:7], in0=ms_all[:, 6:7],
                                        scalar1=0.5, scalar2=None, op0=ALU.mult)
                nc.vector.tensor_scalar(out=e2a[:], in0=e2a[:], scalar1=0.5,
                                        scalar2=None, op0=ALU.mult)
                nc.vector.tensor_scalar(out=ms_all[:, 7:8], in0=s2t[:, 1:2],
                                        scalar1=0.5 / Nh, scalar2=None,
                                        op0=ALU.mult)
                nc.vector.tensor_add(ms_all[:, 7:8], ms_all[:, 7:8], e2a[:])

                # one batched group-norm chain for all 4 cts ([8, 2] per ct)
                gps = gnps.tile([P // GS, 2 * CT], f32, name="gps")
                nc.tensor.matmul(gps[:], lhsT=g16_sb[:], rhs=ms_all[:],
                                 start=True, stop=True)
                gsb = p1s.tile([P // GS, 2 * CT], f32, name="gsb")
                nc.vector.tensor_copy(gsb[:], gps[:])
                gm2 = p1s.tile([P // GS, CT], f32, name="gm2")
                nc.gpsimd.tensor_mul(gm2[:], gsb[:, 0::2], gsb[:, 0::2])
                nc.gpsimd.tensor_sub(gsb[:, 1::2], gsb[:, 1::2], gm2[:])
                # var cols -> 1/sqrt(var + eps)
                nc.scalar.activation(gsb[:, 1::2], gsb[:, 1::2], AF.Sqrt,
                                     bias=eps_sb[:], scale=1.0)
                # dummy exp: swap the act table to the exp set now, while the
                # h writes / u copies run, so the first real exp doesn't pay
                # the ~1.3us LUT load (identity lives in every set)
                nc.scalar.activation(s2t[:, 0:1], s2t[:, 0:1], AF.Exp,
                                     bias=0.0, scale=0.0)
                nc.vector.reciprocal_approx_fast(gsb[:, 1::2], gsb[:, 1::2])
                # broadcast group (mean, rstd) back to 128 channels
                cps = gnps.tile([P, 2 * CT], f32, name="cps")
                nc.tensor.matmul(cps[:], lhsT=gt_sb[:], rhs=gsb[:],
                                 start=True, stop=True)
                scale_all = p1s.tile([P, CT], f32, name="scale_all")
                nc.vector.tensor_mul(scale_all[:], cps[:, 1::2], gamma_sb[:])
                nb1 = p1s.tile([P, CT], f32, name="nb1")
                nc.vector.tensor_mul(nb1[:], cps[:, 0::2], scale_all[:])
                nbias_all = p1s.tile([P, CT], f32, name="nbias_all")
                nc.vector.tensor_sub(nbias_all[:], beta_sb[:], nb1[:])
                warmup(6)
                # h writes, spread across DVE / GpSimd / ScalarE (DVE is
                # fastest per chunk, so it takes 4 of the 8)
                h_engs = (nc.vector, nc.vector, nc.scalar, nc.gpsimd,
                          nc.vector, nc.scalar, nc.gpsimd, nc.vector)
                for ct in range(CT):
                    x_t = x_tiles[ct]
                    for hh in range(2):
                        sl = slice(hh * (N // 2), (hh + 1) * (N // 2))
                        eng = h_engs[2 * ct + hh]
                        if eng is nc.scalar:
                            nc.scalar.activation(
                                h_sb[:, ct, sl], x_t[:, sl], AF.Identity,
                                bias=nbias_all[:, ct:ct + 1],
                                scale=scale_all[:, ct:ct + 1])
                        else:
                            eng.tensor_scalar(
                                out=h_sb[:, ct, sl], in0=x_t[:, sl],
                                scalar1=scale_all[:, ct:ct + 1],
                                scalar2=nbias_all[:, ct:ct + 1],
                                op0=ALU.mult, op1=ALU.add)
                    warmup(4)

            # ---------------- phase B: attention -----------------------------
            with tc.tile_pool(name="ptpool", bufs=JP + 2) as ptp, \
                 tc.tile_pool(name="ops", bufs=1, space="PSUM") as ops, \
                 tc.tile_pool(name="dps", bufs=2, space="PSUM") as dps, \
                 tc.tile_pool(name="dpool", bufs=2) as dpool, \
                 tc.tile_pool(name="mpool", bufs=2) as mp, \
                 tc.tile_pool(name="xrpool", bufs=5) as xrp, \
                 tc.tile_pool(name="outpool", bufs=3) as outp:

                # s-bias scratch: one PSUM bank (shares the o3 bank; ib0's
                # o_tiles are allocated only at first consume, after all
                # s matmuls have been copied out), one column per j-block.
                # Full-bank shape so every o3 allocation is the same size.
                sp = ops.tile([P, FB], f32, name="o3")

                def u_ct(ib, ct, pool, copy_eng):
                    # one [128, FB] chunk of u[:, :, ib-block]: 2 DoubleRow
                    # matmuls + a PSUM copy. JIT chunks borrow the dps 'scr'
                    # banks (idle between final-proj bursts) so the St psum
                    # rotation in sps never waits on a u copy.
                    qp = pool.tile([P, FB], f32,
                                   name="st" if pool is sps else "scr")
                    for kt in range(0, CT, 2):
                        nc.tensor.matmul(
                            qp[:],
                            lhsT=wu_sb[:, kt:kt + 2, ct * P:(ct + 1) * P],
                            rhs=h_sb[:, kt:kt + 2, ib * FB:(ib + 1) * FB],
                            start=(kt == 0), stop=(kt == CT - 2),
                            perf_mode=DR)
                    if copy_eng is nc.scalar:
                        nc.scalar.activation(
                            u_sb[:, ct, ib * FB:(ib + 1) * FB], qp[:],
                            AF.Identity, bias=0.0, scale=1.0)
                    else:
                        copy_eng.tensor_copy(
                            u_sb[:, ct, ib * FB:(ib + 1) * FB], qp[:])

                for ct in range(CT):
                    u_ct(0, ct, sps, nc.vector)

                def final_proj(ib):
                    xrs = []
                    for ct in range(CT):
                        xr = xrp.tile([P, FB], f32, name="xr")
                        nc.sync.dma_start(
                            xr[:], xb_d[ct * P:(ct + 1) * P, ib * FB:(ib + 1) * FB])
                        xrs.append(xr)
                    for ct in range(CT):
                        yp = dps.tile([P, FB], f32, name="scr")
                        for kt in range(0, CT, 2):
                            nc.tensor.matmul(
                                yp[:],
                                lhsT=wp_sb[:, kt:kt + 2, ct * P:(ct + 1) * P],
                                rhs=attn_sb[:, kt:kt + 2, ib * FB:(ib + 1) * FB],
                                start=(kt == 0), stop=(kt == CT - 2),
                                perf_mode=DR)
                        ot = outp.tile([P, FB], f32, name="ot")
                        # y/256 + xb fused: undoes the 16x on vT and 16x on wp
                        nc.vector.scalar_tensor_tensor(
                            out=ot[:], in0=yp[:], scalar=1.0 / 256.0,
                            in1=xrs[ct][:], op0=ALU.mult, op1=ALU.add)
                        nc.sync.dma_start(
                            out_d[ct * P:(ct + 1) * P, ib * FB:(ib + 1) * FB],
                            ot[:])

                for ib in range(NB):
                    o_tiles = []
                    # two independent denominator accumulators halve the
                    # serial DVE chain; bf16 is plenty (errors average out
                    # 1/sqrt(128) in the column-sum matmul) and keeps the den
                    # matmul single-pass
                    dacc = [dpool.tile([P, FB], bf16, name=f"dacc{h}")
                            for h in range(2)]
                    pt_q = []

                    def consume(jp, pt):
                        if jp == 0:
                            # lazy: ib0's vT/s matmuls borrow these banks
                            # during its produce phase
                            o_tiles.extend(ops.tile([P, FB], f32, name=f"o{cs}")
                                           for cs in range(CT))
                        # accumulator 0 on VectorE, accumulator 1 on the
                        # otherwise-idle GpSimd so neither serial chain gates
                        # pt-tile reuse
                        for h, eng in ((0, nc.vector), (1, nc.gpsimd)):
                            if jp == 0:
                                eng.tensor_copy(dacc[h][:], pt[:, h, :])
                            else:
                                eng.tensor_add(dacc[h][:], dacc[h][:],
                                               pt[:, h, :])
                        for cs in range(CT):
                            nc.tensor.matmul(
                                o_tiles[cs][:],
                                lhsT=vt_sb[:, 2 * jp:2 * jp + 2,
                                           cs * P:(cs + 1) * P],
                                rhs=pt[:],
                                start=(jp == 0), stop=(jp == JP - 1),
                                perf_mode=DR)

                    # ib0 also computes vT and the s bias (both share h_j
                    # stationary tiles with St and must lead the O
                    # consumption), so it produces all 16 pairs first and
                    # consumes after; later ibs pipeline with PIPE=5.
                    pipe = JP if ib == 0 else PIPE
                    for jp in range(JP):
                        pt = ptp.tile([P, 2, FB], f8, name="pt")
                        for h in range(2):
                            jb = 2 * jp + h
                            if ib == 0:
                                # s[jb] into column jb of the shared sp bank
                                # FIRST (exp(jb) needs it; the tiny copy must
                                # not queue behind a vt cast on VectorE).
                                # ws carries a 128x host scale (fp8 guard)
                                for kt in range(0, CT, 2):
                                    nc.tensor.matmul(
                                        sp[:, jb:jb + 1],
                                        lhsT=h_sb[:, kt:kt + 2,
                                                  jb * P:(jb + 1) * P],
                                        rhs=ws_sb[:, kt:kt + 2, None],
                                        start=(kt == 0), stop=(kt == CT - 2),
                                        perf_mode=DR, skip_group_check=True)
                                nc.vector.tensor_scalar(
                                    out=st_bias[:, jb:jb + 1],
                                    in0=sp[:, jb:jb + 1],
                                    scalar1=1.0 / 128.0, scalar2=None,
                                    op0=ALU.mult)
                            st = sps.tile([P, FB], f32, name="st")
                            for kt in range(0, CT, 2):
                                nc.tensor.matmul(
                                    st[:],
                                    lhsT=h_sb[:, kt:kt + 2,
                                              jb * P:(jb + 1) * P],
                                    rhs=u_sb[:, kt:kt + 2,
                                             ib * FB:(ib + 1) * FB],
                                    start=(kt == 0), stop=(kt == CT - 2),
                                    perf_mode=DR)
                            # wuT carries a 32x host scale; undo it plus the
                            # 1/sqrt(C) attention scale inside the exp, and add
                            # the per-j softmax bias s[j]
                            nc.scalar.activation(pt[:, h, :], st[:], AF.Exp,
                                                 bias=st_bias[:, jb:jb + 1],
                                                 scale=1.0 / (32.0 * math.sqrt(C)))
                            if ib == 0:
                                # v_j = Wv h_j, transposed layout [j, c];
                                # psum rotates over the idle o0/o1/o2 banks
                                vp = ops.tile([P, C], f32, name=f"o{jb % 3}")
                                for kt in range(0, CT, 2):
                                    nc.tensor.matmul(
                                        vp[:],
                                        lhsT=h_sb[:, kt:kt + 2,
                                                  jb * P:(jb + 1) * P],
                                        rhs=wv_sb[:, kt:kt + 2, :],
                                        start=(kt == 0), stop=(kt == CT - 2),
                                        perf_mode=DR)
                                # split the 32 psum->fp8 casts so VectorE's
                                # FIFO is clear when the consume-phase dacc
                                # chain starts (ScalarE has exp headroom)
                                if jb % 3 == 2:
                                    nc.scalar.activation(
                                        vt_sb[:, jb, :], vp[:], AF.Identity,
                                        bias=0.0, scale=1.0)
                                else:
                                    nc.vector.tensor_copy(vt_sb[:, jb, :],
                                                          vp[:])
                        pt_q.append((jp, pt))
                        if ib > 0:
                            if jp == PIPE:
                                # overlap previous block's output projection
                                # with this block's score matmuls
                                final_proj(ib - 1)
                            if ib < NB - 1 and jp in (8, 10, 12, 14):
                                # next block's u, just-in-time, spread out
                                u_ct(ib + 1, (jp - 8) // 2, dps, nc.vector)
                        if jp >= pipe:
                            consume(*pt_q.pop(0))
                    while pt_q:
                        jp_, pt_ = pt_q.pop(0)
                        consume(jp_, pt_)
                        if ib == 0 and NB > 1 and jp_ in (3, 6, 9, 12):
                            # ScalarE is exp-free during ib0's consume drain
                            u_ct(1, (jp_ // 3) - 1, dps, nc.scalar)

                    # all-ones matmul: every psum partition gets sum_j dacc[j,:]
                    denb = dps.tile([P, FB], f32, name="scr")
                    nc.tensor.matmul(denb[:], lhsT=ones128[:], rhs=dacc[0][:],
                                     start=True, stop=False)
                    nc.tensor.matmul(denb[:], lhsT=ones128[:], rhs=dacc[1][:],
                                     start=False, stop=True)
                    rdb = mp.tile([P, FB], f32, name="rdb")
                    nc.vector.reciprocal_approx_fast(rdb[:], denb[:])
                    for cs in range(CT):
                        nc.vector.tensor_mul(
                            attn_sb[:, cs, ib * FB:(ib + 1) * FB],
                            o_tiles[cs][:], rdb[:])
                final_proj(NB - 1)

    nc.compile()
    return nc


def _host_inputs(x, gamma, beta, Wq, bq, Wk, bk, Wv, bv, Wp, bp):
    bf16 = ml_dtypes.bfloat16
    f32 = np.float32
    B = x.shape[0]
    xs = np.asarray(x, f32).reshape(B, C, N)

    def fold(v):
        return np.asarray(v, f32).reshape(CT, P).T.copy()

    f8 = ml_dtypes.float8_e4m3fn

    def wtile(w, scale, dt):
        # [Cout, Cin] -> transposed [Cin, Cout] -> tiled [P, CT, Cout]
        wT = np.asarray(w, f32).T * scale
        return np.ascontiguousarray(
            wT.reshape(CT, P, C).transpose(1, 0, 2)).astype(dt)

    # folded score matrix: softmax_j(q_i.k_j/sqrt(C)) with q=Wq h+bq,
    # k=Wk h+bk equals softmax_j(h_j.(M h_i)/sqrt(C) + s_j) with
    # M = Wk^T Wq and s = (Wk^T bq).h_j/sqrt(C); bk and i-only terms cancel.
    M = np.asarray(Wk, f32).T @ np.asarray(Wq, f32)
    wsv = (np.asarray(Wk, f32).T @ np.asarray(bq, f32)) / math.sqrt(C)
    common = {
        "wuT": wtile(M, 32.0, f8),
        "wvT": wtile(Wv, 16.0, f8),
        "wpT": wtile(Wp, 16.0, f8),
        "ws": (wsv * 128.0).reshape(CT, P).T.copy().astype(f8),
        "gamma": fold(gamma),
        "beta": fold(beta),
    }
    bias_out = (np.asarray(Wp, f32) @ np.asarray(bv, f32)
                + np.asarray(bp, f32)).astype(f32)
    xbs = xs + bias_out[None, :, None]
    g16 = np.zeros((P, P // GS), f32)
    g16[np.arange(P), np.arange(P) // GS] = 1.0 / GS
    gt = np.zeros((P // GS, P), f32)
    gt[np.arange(P) // GS, np.arange(P)] = 1.0
    common["g16"] = g16
    common["gt"] = gt
    return [dict(common, xh=np.ascontiguousarray(xs[b]).astype(f8),
                 xb=np.ascontiguousarray(xbs[b])) for b in range(B)]


def kernel(x, gamma, beta, Wq, bq, Wk, bk, Wv, bv, Wp, bp, _trace=False):
    from concourse.bass_utils import run_bass_kernel_spmd

    if "nc" not in _CACHE:
        _CACHE["nc"] = _build()
    nc = _CACHE["nc"]
    in_maps = _host_inputs(x, gamma, beta, Wq, bq, Wk, bk, Wv, bv, Wp, bp)
    B = len(in_maps)
    res = run_bass_kernel_spmd(nc, in_maps, core_ids=list(range(B)),
                               trace=_trace)
    out = np.stack([res.results[b]["out"] for b in range(B)])
    out = out.reshape(x.shape).astype(np.float32)
    if _trace:
        _CACHE["last_results"] = res
    return out
